# revision 1
# baseline (speedup 1.0000x reference)
"""DeepSeek-V3 MoE block on 8 trn2 NeuronCores.

Expert-parallel sparse MoE:
  - host computes routing indices (dispatch metadata) in fp32 numpy
  - experts sorted by token count into 4 tiers of 8; core c slot k holds the
    (8k+c)-th busiest expert; slot capacity = padded max count of its tier so
    every core runs the identical program (SPMD) with zero weight duplication
  - device computes: gate logits (true fp32 matmul), sigmoid+grouped-top-k
    routing weights, per-expert gated MLP (fp32r matmuls), shared-expert MLP
    (intermediate dim sharded 8-way), combine-scaling at the down projection
  - host sums the 8 shared-expert partials and scatter-adds the routed rows
"""

import os
import sys

sys.path.insert(0, "/opt/trn_rl_repo")

import numpy as np

import concourse.bacc as bacc
import concourse.bass as bass
import concourse.mybir as mybir
import concourse.tile as tile
from concourse.bass_utils import run_bass_kernel_spmd

F32 = mybir.dt.float32
F32R = mybir.dt.float32r
AF = mybir.ActivationFunctionType
ALU = mybir.AluOpType
AX = mybir.AxisListType

T, H, I, IS, E = 1024, 1024, 512, 2048, 32
G, TOPK_GROUP, TOP_K = 8, 4, 8
SCALE = 2.5
NCORES = 8
S = E // NCORES          # expert slots per core
ISH = IS // NCORES       # shared-expert intermediate shard
P128 = 128

LAST_RESULTS = None      # BassKernelResults of the most recent run


def _install_ntff_hook():
    """Provide antenv.axon_hooks + the ctypes NTFF profile hook when the
    container image lacks them (needed only for trace=True)."""
    import contextlib
    import ctypes
    import types

    try:
        from antenv.axon_hooks import get_axon_ntff_profile_hook  # noqa: F401
        return True
    except ImportError:
        pass
    try:
        import antenv
        so_path = "/opt/axon/libaxon_pjrt.so"
        lib = ctypes.CDLL(so_path)
        if not hasattr(lib, "axon_start_nrt_profile"):
            return False
        lib.axon_start_nrt_profile.argtypes = [
            ctypes.POINTER(ctypes.c_int64), ctypes.c_size_t]
        lib.axon_start_nrt_profile.restype = ctypes.c_int64
        lib.axon_stop_nrt_profile.argtypes = [ctypes.c_char_p]
        lib.axon_stop_nrt_profile.restype = ctypes.c_int64

        @contextlib.contextmanager
        def _hook(output_dir, device_ids):
            import jax
            jax.devices()
            if device_ids:
                ids = (ctypes.c_int64 * len(device_ids))(*device_ids)
                rc = lib.axon_start_nrt_profile(ids, len(device_ids))
            else:
                rc = lib.axon_start_nrt_profile(None, 0)
            if rc != 0:
                raise RuntimeError(f"axon_start_nrt_profile rc={rc}")
            try:
                yield
            finally:
                n = lib.axon_stop_nrt_profile(str(output_dir).encode())
                print(f"ntff profile: {n} file(s) -> {output_dir}",
                      file=sys.stderr)

        mod = types.ModuleType("antenv.axon_hooks")
        _state = {"hook": _hook}
        mod.set_axon_ntff_profile_hook = lambda h: _state.__setitem__("hook", h)
        mod.get_axon_ntff_profile_hook = lambda: _state["hook"]
        sys.modules["antenv.axon_hooks"] = mod
        antenv.axon_hooks = mod
        return True
    except Exception:
        return False


def _host_routing(x, gate_w, e_bias):
    """fp32 numpy mirror of reference._routing_combine; returns emask [T,E]."""
    logits = x.astype(np.float32) @ gate_w.T.astype(np.float32)
    scores = 1.0 / (1.0 + np.exp(-logits))
    swb = scores + e_bias[None, :]
    swb_g = swb.reshape(T, G, E // G)
    gs = np.sort(swb_g, axis=-1)[..., -2:].sum(-1)          # top-2 sum per group
    thr4 = np.sort(gs, axis=-1)[:, -TOPK_GROUP][:, None]
    gmask = (gs >= thr4).astype(np.float32)
    smask = np.repeat(gmask, E // G, axis=-1)
    masked = swb * smask
    thr8 = np.sort(masked, axis=-1)[:, -TOP_K][:, None]
    emask = masked >= thr8
    return emask


def _pad128(n):
    # floor 256: fp32r matmuls need moving dim >= 256 for full PE rate
    return max(256, ((int(n) + P128 - 1) // P128) * P128)


def _chunks(p, limit=512):
    """Split width p into chunks <= limit (each a multiple of 128)."""
    out = []
    o = 0
    while o < p:
        w = min(limit, p - o)
        out.append((o, w))
        o += w
    return out


def _build_program(P):
    """Emit the SPMD Bass program for slot capacities P (list of S ints)."""
    nc = bacc.Bacc(target_bir_lowering=False, debug=False)
    max_nc = P[0] // P128

    # ---- DRAM parameters (per-core data arrives via in_maps) ----
    xe_d = [nc.dram_tensor(f"xe{k}", [H, P[k]], F32R, kind="ExternalInput")
            for k in range(S)]
    wg_d = [nc.dram_tensor(f"wg{k}", [H, I], F32R, kind="ExternalInput")
            for k in range(S)]
    wu_d = [nc.dram_tensor(f"wu{k}", [H, I], F32R, kind="ExternalInput")
            for k in range(S)]
    wd_d = [nc.dram_tensor(f"wd{k}", [I, H], F32R, kind="ExternalInput")
            for k in range(S)]
    oh_d = [nc.dram_tensor(f"oh{k}", [P128, (P[k] // P128) * E], F32,
                           kind="ExternalInput") for k in range(S)]
    em_d = [nc.dram_tensor(f"em{k}", [P128, (P[k] // P128) * E], F32,
                           kind="ExternalInput") for k in range(S)]
    gwt_d = nc.dram_tensor("gwt", [P128, H // P128, E], F32R, kind="ExternalInput")
    xt_d = nc.dram_tensor("xt", [H, T], F32R, kind="ExternalInput")
    wsg_d = nc.dram_tensor("wsg", [H, ISH], F32R, kind="ExternalInput")
    wsu_d = nc.dram_tensor("wsu", [H, ISH], F32R, kind="ExternalInput")
    wsd_d = nc.dram_tensor("wsd", [ISH, H], F32R, kind="ExternalInput")
    ro_d = [nc.dram_tensor(f"ro{k}", [P[k], H], F32, kind="ExternalOutput")
            for k in range(S)]
    so_d = nc.dram_tensor("so", [T, H], F32, kind="ExternalOutput")

    HT = H // P128  # 8 h-tiles

    with tile.TileContext(nc) as tc:
        with (
            tc.tile_pool(name="const", bufs=1) as cpool,
            tc.tile_pool(name="xe", bufs=8) as xepool,
            tc.tile_pool(name="w", bufs=10) as wpool,
            tc.tile_pool(name="wd", bufs=5) as wdpool,
            tc.tile_pool(name="acts", bufs=5) as apool,
            tc.tile_pool(name="rt", bufs=1) as rpool,
            tc.tile_pool(name="scl", bufs=1) as spool,
            tc.tile_pool(name="stage", bufs=4) as stpool,
            tc.tile_pool(name="ps", bufs=2, space="PSUM") as ps,
        ):
            # ---- constants ----
            gwt = cpool.tile([P128, HT, E], F32R, tag="gwt")
            nc.sync.dma_start(out=gwt[:], in_=gwt_d[:])

            # ---- per-slot: DMA (consumption order) + routing + MLP ----
            scale_sb = []
            xt_sb = []
            for k in range(S):
                n_c = P[k] // P128
                oh_t = cpool.tile([P128, n_c * E], F32, tag=f"oh{k}", name="oht")
                nc.sync.dma_start(out=oh_t[:], in_=oh_d[k][:])
                em_t = cpool.tile([P128, n_c * E], F32, tag=f"em{k}", name="emt")
                nc.sync.dma_start(out=em_t[:], in_=em_d[k][:])
                xs = []
                for h in range(HT):
                    xt_t = xepool.tile([P128, P[k]], F32R, tag=f"xe{k}", name="xet")
                    nc.sync.dma_start(out=xt_t[:],
                                      in_=xe_d[k][h * P128:(h + 1) * P128, :])
                    xs.append(xt_t)
                wg_t, wu_t = [], []
                for h in range(HT):
                    a = wpool.tile([P128, I], F32R, tag="wg", bufs=12, name="wgt")
                    nc.sync.dma_start(out=a[:],
                                      in_=wg_d[k][h * P128:(h + 1) * P128, :])
                    wg_t.append(a)
                    b = wpool.tile([P128, I], F32R, tag="wu", bufs=12, name="wut")
                    nc.sync.dma_start(out=b[:],
                                      in_=wu_d[k][h * P128:(h + 1) * P128, :])
                    wu_t.append(b)
                wd_t = []
                for ii in range(I // P128):
                    dd = wdpool.tile([P128, H], F32R, tag="wd", bufs=6, name="wdt")
                    nc.sync.dma_start(out=dd[:],
                                      in_=wd_d[k][ii * P128:(ii + 1) * P128, :])
                    wd_t.append(dd)

                # spread the shared-expert xt stream across slot boundaries
                for _j in range(HT // S):
                    t = xepool.tile([P128, T], F32R, tag="xt", name="xtt")
                    nc.sync.dma_start(
                        out=t[:],
                        in_=xt_d[len(xt_sb) * P128:(len(xt_sb) + 1) * P128, :])
                    xt_sb.append(t)

                # logits for this slot's gathered tokens
                lg = ps.tile([P128, n_c * E], F32, tag="lg")
                for cc in range(n_c):
                    for h in range(HT):
                        nc.tensor.matmul(
                            lg[:, cc * E:(cc + 1) * E],
                            lhsT=xs[h][:, cc * P128:(cc + 1) * P128],
                            rhs=gwt[:, h, :],
                            start=(h == 0), stop=(h == HT - 1),
                        )
                # combine weights: sigmoid scores; selection mask comes
                # from the host dispatch (bit-identical to reference topk)
                scores = rpool.tile([P128, n_c * E], F32, tag="scores")
                nc.scalar.activation(scores[:], lg[:], AF.Sigmoid)
                sct = rpool.tile([P128, n_c * E], F32, tag="sct")
                nc.vector.tensor_mul(sct[:], scores[:], em_t[:])
                den = rpool.tile([P128, n_c], F32, tag="den")
                nc.vector.tensor_reduce(
                    den[:], sct[:].rearrange("p (c e) -> p c e", e=E),
                    axis=AX.X, op=ALU.add)
                num_t = scores  # in-place: scores dead after sct
                nc.vector.tensor_mul(num_t[:], scores[:], oh_t[:])
                num = rpool.tile([P128, n_c], F32, tag="num")
                nc.vector.tensor_reduce(
                    num[:], num_t[:].rearrange("p (c e) -> p c e", e=E),
                    axis=AX.X, op=ALU.add)
                rec = rpool.tile([P128, n_c], F32, tag="rec")
                nc.vector.reciprocal(rec[:], den[:])
                scl = spool.tile([P128, n_c], F32, tag=f"scale{k}")
                nc.vector.tensor_mul(scl[:], num[:], rec[:])
                nc.vector.tensor_scalar_mul(scl[:], scl[:], SCALE)
                scale_sb.append(scl)

                # expert MLP
                acts = [apool.tile([P128, P[k]], F32R, tag="acts", bufs=4,
                                   name=f"acts{ii}") for ii in range(I // P128)]
                for ii in range(I // P128):
                    for (mo, mw) in _chunks(P[k]):
                        h1 = ps.tile([P128, mw], F32, tag="h1")
                        h2 = ps.tile([P128, mw], F32, tag="h2")
                        for h in range(HT):
                            nc.tensor.matmul(
                                h1[:], lhsT=wg_t[h][:, ii * P128:(ii + 1) * P128],
                                rhs=xs[h][:, mo:mo + mw],
                                start=(h == 0), stop=(h == HT - 1))
                        for h in range(HT):
                            nc.tensor.matmul(
                                h2[:], lhsT=wu_t[h][:, ii * P128:(ii + 1) * P128],
                                rhs=xs[h][:, mo:mo + mw],
                                start=(h == 0), stop=(h == HT - 1))
                        sl = stpool.tile([P128, mw], F32, tag="silu", bufs=3,
                                         name="sl")
                        nc.scalar.activation(sl[:], h1[:], AF.Sigmoid)
                        nc.vector.tensor_mul(sl[:], sl[:], h1[:])
                        nc.vector.tensor_mul(acts[ii][:, mo:mo + mw], sl[:], h2[:])

                for cc in range(n_c):
                    for hh in range(2):
                        dps = ps.tile([P128, H // 2], F32, tag="dps")
                        for ii in range(I // P128):
                            nc.tensor.matmul(
                                dps[:],
                                lhsT=acts[ii][:, cc * P128:(cc + 1) * P128],
                                rhs=wd_t[ii][:, hh * (H // 2):(hh + 1) * (H // 2)],
                                start=(ii == 0), stop=(ii == I // P128 - 1))
                        ost = stpool.tile([P128, H // 2], F32, tag="ost")
                        nc.vector.tensor_scalar_mul(ost[:], dps[:],
                                                    scale_sb[k][:, cc:cc + 1])
                        nc.gpsimd.dma_start(
                            out=ro_d[k][cc * P128:(cc + 1) * P128,
                                        hh * (H // 2):(hh + 1) * (H // 2)],
                            in_=ost[:])

            # ---- shared expert (intermediate shard ISH=256) ----
            wsg_t, wsu_t = [], []
            for h in range(HT):
                a = wpool.tile([P128, ISH], F32R, tag="wg", bufs=12, name="wsgt")
                nc.sync.dma_start(out=a[:], in_=wsg_d[h * P128:(h + 1) * P128, :])
                wsg_t.append(a)
                b = wpool.tile([P128, ISH], F32R, tag="wu", bufs=12, name="wsut")
                nc.sync.dma_start(out=b[:], in_=wsu_d[h * P128:(h + 1) * P128, :])
                wsu_t.append(b)
            wsd_t = []
            for ii in range(ISH // P128):
                d = wdpool.tile([P128, H], F32R, tag="wd", bufs=6, name="wsdt")
                nc.sync.dma_start(out=d[:], in_=wsd_d[ii * P128:(ii + 1) * P128, :])
                wsd_t.append(d)

            acts_s = [apool.tile([P128, T], F32R, tag="acts_s", bufs=2, name=f"acts_s{ii}")
                      for ii in range(ISH // P128)]
            for ii in range(ISH // P128):
                for (mo, mw) in _chunks(T):
                    h1 = ps.tile([P128, mw], F32, tag="h1")
                    h2 = ps.tile([P128, mw], F32, tag="h2")
                    for h in range(HT):
                        nc.tensor.matmul(
                            h1[:], lhsT=wsg_t[h][:, ii * P128:(ii + 1) * P128],
                            rhs=xt_sb[h][:, mo:mo + mw],
                            start=(h == 0), stop=(h == HT - 1))
                    for h in range(HT):
                        nc.tensor.matmul(
                            h2[:], lhsT=wsu_t[h][:, ii * P128:(ii + 1) * P128],
                            rhs=xt_sb[h][:, mo:mo + mw],
                            start=(h == 0), stop=(h == HT - 1))
                    sl = stpool.tile([P128, mw], F32, tag="silu", bufs=3, name="sl")
                    nc.scalar.activation(sl[:], h1[:], AF.Sigmoid)
                    nc.vector.tensor_mul(sl[:], sl[:], h1[:])
                    nc.vector.tensor_mul(acts_s[ii][:, mo:mo + mw], sl[:], h2[:])

            for cc in range(T // P128):
                for hh in range(2):
                    dps = ps.tile([P128, H // 2], F32, tag="dps")
                    for ii in range(ISH // P128):
                        nc.tensor.matmul(
                            dps[:],
                            lhsT=acts_s[ii][:, cc * P128:(cc + 1) * P128],
                            rhs=wsd_t[ii][:, hh * (H // 2):(hh + 1) * (H // 2)],
                            start=(ii == 0), stop=(ii == ISH // P128 - 1))
                    ost = stpool.tile([P128, H // 2], F32, tag="ost")
                    nc.vector.tensor_copy(ost[:], dps[:])
                    nc.gpsimd.dma_start(
                        out=so_d[cc * P128:(cc + 1) * P128,
                                 hh * (H // 2):(hh + 1) * (H // 2)],
                        in_=ost[:])

    nc.compile()
    return nc


def _prepare(inputs):
    """Host-side dispatch prep: returns (in_maps, P, slot_expert, tok_lists)."""
    x = np.ascontiguousarray(inputs["hidden_states"], dtype=np.float32)
    gate_w = np.asarray(inputs["gate_w"], dtype=np.float32)
    e_bias = np.asarray(inputs["e_bias"], dtype=np.float32)
    w_gate = np.asarray(inputs["w_gate"], dtype=np.float32)
    w_up = np.asarray(inputs["w_up"], dtype=np.float32)
    w_down = np.asarray(inputs["w_down"], dtype=np.float32)
    ws_gate = np.asarray(inputs["ws_gate"], dtype=np.float32)
    ws_up = np.asarray(inputs["ws_up"], dtype=np.float32)
    ws_down = np.asarray(inputs["ws_down"], dtype=np.float32)

    # ---- dispatch metadata ----
    emask = _host_routing(x, gate_w, e_bias)
    counts = emask.sum(0).astype(np.int64)
    order = np.argsort(-counts, kind="stable")
    tok_lists = [np.nonzero(emask[:, e])[0] for e in range(E)]
    # slot k of every core serves tier k. Order: 2nd-biggest tier first (its
    # inputs land fast so the PE starts early), biggest second (its long
    # compute overlaps the remaining input stream), then the small tiers.
    tiers = [[int(order[k * NCORES + c]) for c in range(NCORES)]
             for k in range(S)]
    # descending tier order measured fastest: the big slot's inputs stream
    # while the routing/logits warm up, and small slots + shared fill the tail
    P = [_pad128(max(counts[e] for e in tier)) for tier in tiers]
    max_nc = P[0] // P128

    xt = np.ascontiguousarray(x.T)
    gwt = np.ascontiguousarray(
        gate_w.T.reshape(H // P128, P128, E).transpose(1, 0, 2))
    in_maps = []
    slot_expert = np.zeros((NCORES, S), dtype=np.int64)
    emf = emask.astype(np.float32)
    for c in range(NCORES):
        m = {"gwt": gwt, "xt": xt,
             "wsg": np.ascontiguousarray(ws_gate[:, c * ISH:(c + 1) * ISH]),
             "wsu": np.ascontiguousarray(ws_up[:, c * ISH:(c + 1) * ISH]),
             "wsd": np.ascontiguousarray(ws_down[c * ISH:(c + 1) * ISH, :])}
        for k in range(S):
            e = tiers[k][c]
            slot_expert[c, k] = e
            toks = tok_lists[e]
            xe = np.zeros((H, P[k]), dtype=np.float32)
            xe[:, :len(toks)] = x[toks].T
            n_c = P[k] // P128
            oh = np.zeros((P128, n_c * E), dtype=np.float32)
            oh[:, e::E] = 1.0
            em = np.ones((n_c * P128, E), dtype=np.float32)
            em[:len(toks)] = emf[toks]
            em = np.ascontiguousarray(
                em.reshape(n_c, P128, E).transpose(1, 0, 2).reshape(P128, n_c * E))
            m[f"xe{k}"] = xe
            m[f"wg{k}"] = np.ascontiguousarray(w_gate[e])
            m[f"wu{k}"] = np.ascontiguousarray(w_up[e])
            m[f"wd{k}"] = np.ascontiguousarray(w_down[e])
            m[f"oh{k}"] = oh
            m[f"em{k}"] = em
        in_maps.append(m)

    return in_maps, P, slot_expert, tok_lists


def _recombine(results, slot_expert, tok_lists):
    out = np.zeros((T, H), dtype=np.float32)
    for c in range(NCORES):
        out += results[c]["so"]
    for c in range(NCORES):
        for k in range(S):
            e = slot_expert[c, k]
            toks = tok_lists[e]
            out[toks] += results[c][f"ro{k}"][:len(toks)]
    return out


def kernel(**inputs):
    global LAST_RESULTS
    in_maps, P, slot_expert, tok_lists = _prepare(inputs)
    nc = _build_program(P)
    trace = bool(int(os.environ.get("KERNEL_TRACE", "0")))
    if trace:
        trace = _install_ntff_hook()
    LAST_RESULTS = run_bass_kernel_spmd(
        nc, in_maps, list(range(NCORES)), trace=trace)
    results = LAST_RESULTS.results
    return _recombine(results, slot_expert, tok_lists)



# revision 2
# speedup vs baseline: 1.7030x; 1.7030x over previous
"""DeepSeek-V3 MoE block on 8 trn2 NeuronCores.

Expert-parallel sparse MoE, bf16 datapath (fp32 PSUM accumulation):
  - host computes routing + combine weights in fp32 numpy (dispatch
    metadata, bit-matched to the reference's sigmoid/top-k math)
  - experts sorted by token count into 4 tiers of 8; core c slot k holds the
    (8k+c)-th busiest expert; slot capacity = that tier's max count (exact,
    rounded to 8) so every core runs the identical program (SPMD)
  - per-slot inputs (gate/up weights, gathered tokens, down weights) are
    host-packed into one [128, W] bf16 DRAM tensor, moved by a handful of
    large DMAs (>=0.4 MB each) ordered so the PE streams behind the DMA
  - device computes: per-expert gated MLP (bf16 matmuls, AF.Silu on the
    scalar engine), combine-scaling fused into the PSUM->SBUF copy via
    activation(Copy, scale=AP), shared-expert MLP with the intermediate
    dim sharded 8-way
  - outputs are bf16; host sums the 8 shared-expert partials and
    scatter-adds the routed rows in fp32
"""

import os
import sys

sys.path.insert(0, "/opt/trn_rl_repo")

import numpy as np
import ml_dtypes

import concourse.bacc as bacc
import concourse.bass as bass
import concourse.mybir as mybir
import concourse.tile as tile
from concourse.bass_utils import run_bass_kernel_spmd

F32 = mybir.dt.float32
BF16 = mybir.dt.bfloat16
AF = mybir.ActivationFunctionType

BF = ml_dtypes.bfloat16

T, H, I, IS, E = 1024, 1024, 512, 2048, 32
G, TOPK_GROUP, TOP_K = 8, 4, 8
SCALE = 2.5
NCORES = 8
S = E // NCORES          # expert slots per core
ISH = IS // NCORES       # shared-expert intermediate shard
P128 = 128
HT = H // P128           # 8 h-tiles

LAST_RESULTS = None      # BassKernelResults of the most recent run


def _install_ntff_hook():
    """Provide antenv.axon_hooks + the ctypes NTFF profile hook when the
    container image lacks them (needed only for trace=True)."""
    import contextlib
    import ctypes
    import types

    try:
        from antenv.axon_hooks import get_axon_ntff_profile_hook  # noqa: F401
        return True
    except ImportError:
        pass
    try:
        import antenv
        so_path = "/opt/axon/libaxon_pjrt.so"
        lib = ctypes.CDLL(so_path)
        if not hasattr(lib, "axon_start_nrt_profile"):
            return False
        lib.axon_start_nrt_profile.argtypes = [
            ctypes.POINTER(ctypes.c_int64), ctypes.c_size_t]
        lib.axon_start_nrt_profile.restype = ctypes.c_int64
        lib.axon_stop_nrt_profile.argtypes = [ctypes.c_char_p]
        lib.axon_stop_nrt_profile.restype = ctypes.c_int64

        @contextlib.contextmanager
        def _hook(output_dir, device_ids):
            import jax
            jax.devices()
            if device_ids:
                ids = (ctypes.c_int64 * len(device_ids))(*device_ids)
                rc = lib.axon_start_nrt_profile(ids, len(device_ids))
            else:
                rc = lib.axon_start_nrt_profile(None, 0)
            if rc != 0:
                raise RuntimeError(f"axon_start_nrt_profile rc={rc}")
            try:
                yield
            finally:
                n = lib.axon_stop_nrt_profile(str(output_dir).encode())
                print(f"ntff profile: {n} file(s) -> {output_dir}",
                      file=sys.stderr)

        mod = types.ModuleType("antenv.axon_hooks")
        _state = {"hook": _hook}
        mod.set_axon_ntff_profile_hook = lambda h: _state.__setitem__("hook", h)
        mod.get_axon_ntff_profile_hook = lambda: _state["hook"]
        sys.modules["antenv.axon_hooks"] = mod
        antenv.axon_hooks = mod
        return True
    except Exception:
        return False


def _host_routing(x, gate_w, e_bias):
    """fp32 numpy mirror of reference._routing_combine.

    Returns (emask [T,E] bool, comb [T,E] fp32 combine weights)."""
    logits = x.astype(np.float32) @ gate_w.T.astype(np.float32)
    scores = 1.0 / (1.0 + np.exp(-logits))
    swb = scores + e_bias[None, :]
    swb_g = swb.reshape(T, G, E // G)
    gs = np.sort(swb_g, axis=-1)[..., -2:].sum(-1)          # top-2 sum per group
    thr4 = np.sort(gs, axis=-1)[:, -TOPK_GROUP][:, None]
    gmask = (gs >= thr4).astype(np.float32)
    smask = np.repeat(gmask, E // G, axis=-1)
    masked = swb * smask
    thr8 = np.sort(masked, axis=-1)[:, -TOP_K][:, None]
    emask = masked >= thr8
    sc = scores * emask
    comb = sc / (sc.sum(-1, keepdims=True) + 1e-20) * SCALE
    return emask, comb


def _pad8(n):
    return max(16, ((int(n) + 7) // 8) * 8)


def _chunks(p, limit=512):
    out = []
    o = 0
    while o < p:
        w = min(limit, p - o)
        out.append((o, w))
        o += w
    return out


def _build_program(P):
    """Emit the SPMD Bass program for slot capacities P (list of S ints)."""
    nc = bacc.Bacc(target_bir_lowering=False, debug=False)

    NCC = [(P[k] + P128 - 1) // P128 for k in range(S)]   # 128-token blocks
    COL = [sum(NCC[:k]) for k in range(S)]                # scale column offset

    # per-slot packed input: 8 h-blocks of [wg_h(512) | wu_h(512) | xe_h(P)]
    # followed by 4 wd i-blocks of [128, 1024]
    BL = [HT * P128 + P[k] for k in range(S)]             # h-block width
    W = [HT * BL[k] + (I // P128) * H for k in range(S)]

    mg_d = [nc.dram_tensor(f"mg{k}", [P128, W[k]], BF16, kind="ExternalInput")
            for k in range(S)]
    scl_d = nc.dram_tensor("scl", [P128, sum(NCC)], F32, kind="ExternalInput")
    # shared expert: [wsg_h(256)|wsu_h(256)] x8, xt h-blocks, wsd i-blocks
    wsgu_d = nc.dram_tensor("wsgu", [P128, HT * 2 * ISH], BF16,
                            kind="ExternalInput")
    xt_d = nc.dram_tensor("xt", [P128, HT * T], BF16, kind="ExternalInput")
    wsd_d = nc.dram_tensor("wsd", [P128, (ISH // P128) * H], BF16,
                           kind="ExternalInput")
    ro_d = [nc.dram_tensor(f"ro{k}", [P[k], H], BF16, kind="ExternalOutput")
            for k in range(S)]
    so_d = nc.dram_tensor("so", [T, H], BF16, kind="ExternalOutput")

    with tile.TileContext(nc) as tc:
        with (
            tc.tile_pool(name="const", bufs=1) as cpool,
            tc.tile_pool(name="acts", bufs=2) as apool,
            tc.tile_pool(name="stage", bufs=3) as stpool,
            tc.tile_pool(name="ps", bufs=2, space="PSUM") as ps,
            tc.tile_pool(name="psd", bufs=3, space="PSUM") as psd,
        ):
            # ---- input DMAs, in arrival order (single HWDGE FIFO ring) ----
            scl_t = cpool.tile([P128, sum(NCC)], F32, tag="scl")
            nc.sync.dma_start(out=scl_t[:], in_=scl_d[:])

            mg_t = []
            for k in range(S):
                m = cpool.tile([P128, W[k]], BF16, tag=f"mg{k}", name="mgt")
                mg_t.append(m)
                if k == 0:
                    # fine-grained chunks: PE starts as soon as h-block 0 lands
                    for h in range(HT):
                        nc.sync.dma_start(
                            out=m[:, h * BL[k]:(h + 1) * BL[k]],
                            in_=mg_d[k][:, h * BL[k]:(h + 1) * BL[k]])
                    nc.sync.dma_start(out=m[:, HT * BL[k]:],
                                      in_=mg_d[k][:, HT * BL[k]:])
                else:
                    nc.sync.dma_start(out=m[:, :HT * BL[k]],
                                      in_=mg_d[k][:, :HT * BL[k]])
                    nc.sync.dma_start(out=m[:, HT * BL[k]:],
                                      in_=mg_d[k][:, HT * BL[k]:])

            wsgu_t = cpool.tile([P128, HT * 2 * ISH], BF16, tag="wsgu")
            half = HT * ISH
            nc.sync.dma_start(out=wsgu_t[:, :half], in_=wsgu_d[:, :half])
            nc.sync.dma_start(out=wsgu_t[:, half:], in_=wsgu_d[:, half:])
            xt_t = cpool.tile([P128, HT * T], BF16, tag="xt")
            for q in range(4):
                w4 = HT * T // 4
                nc.sync.dma_start(out=xt_t[:, q * w4:(q + 1) * w4],
                                  in_=xt_d[:, q * w4:(q + 1) * w4])
            wsd_t = cpool.tile([P128, (ISH // P128) * H], BF16, tag="wsd")
            nc.sync.dma_start(out=wsd_t[:], in_=wsd_d[:])

            # ---- routed experts ----
            for k in range(S):
                m = mg_t[k]
                wdoff = HT * BL[k]
                acts = [apool.tile([P128, P[k]], BF16, tag="acts", bufs=8,
                                   name=f"a{k}_{ii}") for ii in range(I // P128)]
                for ii in range(I // P128):
                    for (mo, mw) in _chunks(P[k]):
                        h1 = ps.tile([P128, mw], F32, tag="h1")
                        h2 = ps.tile([P128, mw], F32, tag="h2")
                        for h in range(HT):
                            nc.tensor.matmul(
                                h1[:],
                                lhsT=m[:, h * BL[k] + ii * P128:
                                       h * BL[k] + (ii + 1) * P128],
                                rhs=m[:, h * BL[k] + HT * P128 + mo:
                                      h * BL[k] + HT * P128 + mo + mw],
                                start=(h == 0), stop=(h == HT - 1))
                        for h in range(HT):
                            nc.tensor.matmul(
                                h2[:],
                                lhsT=m[:, h * BL[k] + I + ii * P128:
                                       h * BL[k] + I + (ii + 1) * P128],
                                rhs=m[:, h * BL[k] + HT * P128 + mo:
                                      h * BL[k] + HT * P128 + mo + mw],
                                start=(h == 0), stop=(h == HT - 1))
                        sl = stpool.tile([P128, mw], F32, tag="sl", name="sl")
                        nc.scalar.activation(sl[:], h1[:], AF.Silu)
                        nc.vector.tensor_mul(acts[ii][:, mo:mo + mw],
                                             sl[:], h2[:])

                for cc in range(NCC[k]):
                    pw = min(P128, P[k] - cc * P128)
                    ost = stpool.tile([P128, H], BF16, tag="ost", name="ost")
                    for hh in range(2):
                        dps = psd.tile([P128, H // 2], F32, tag="dps")
                        for ii in range(I // P128):
                            nc.tensor.matmul(
                                dps[:pw, :],
                                lhsT=acts[ii][:, cc * P128:cc * P128 + pw],
                                rhs=m[:, wdoff + ii * H + hh * (H // 2):
                                      wdoff + ii * H + (hh + 1) * (H // 2)],
                                start=(ii == 0), stop=(ii == I // P128 - 1))
                        # fused combine-scale + bf16 downcast on the ACT engine
                        nc.scalar.activation(
                            ost[:pw, hh * (H // 2):(hh + 1) * (H // 2)],
                            dps[:pw, :], AF.Copy,
                            scale=scl_t[:pw, COL[k] + cc:COL[k] + cc + 1])
                    nc.gpsimd.dma_start(
                        out=ro_d[k][cc * P128:cc * P128 + pw, :],
                        in_=ost[:pw, :])

            # ---- shared expert (intermediate shard ISH=256) ----
            acts_s = [apool.tile([P128, T], BF16, tag="acts", bufs=8,
                                 name=f"as{ii}") for ii in range(ISH // P128)]
            for ii in range(ISH // P128):
                for (mo, mw) in _chunks(T):
                    h1 = ps.tile([P128, mw], F32, tag="h1")
                    h2 = ps.tile([P128, mw], F32, tag="h2")
                    for h in range(HT):
                        nc.tensor.matmul(
                            h1[:],
                            lhsT=wsgu_t[:, h * 2 * ISH + ii * P128:
                                        h * 2 * ISH + (ii + 1) * P128],
                            rhs=xt_t[:, h * T + mo:h * T + mo + mw],
                            start=(h == 0), stop=(h == HT - 1))
                    for h in range(HT):
                        nc.tensor.matmul(
                            h2[:],
                            lhsT=wsgu_t[:, h * 2 * ISH + ISH + ii * P128:
                                        h * 2 * ISH + ISH + (ii + 1) * P128],
                            rhs=xt_t[:, h * T + mo:h * T + mo + mw],
                            start=(h == 0), stop=(h == HT - 1))
                    sl = stpool.tile([P128, mw], F32, tag="sl", name="sl")
                    nc.scalar.activation(sl[:], h1[:], AF.Silu)
                    nc.vector.tensor_mul(acts_s[ii][:, mo:mo + mw],
                                         sl[:], h2[:])

            for cc in range(T // P128):
                ost = stpool.tile([P128, H], BF16, tag="ost", name="ost")
                for hh in range(2):
                    dps = psd.tile([P128, H // 2], F32, tag="dps")
                    for ii in range(ISH // P128):
                        nc.tensor.matmul(
                            dps[:],
                            lhsT=acts_s[ii][:, cc * P128:(cc + 1) * P128],
                            rhs=wsd_t[:, ii * H + hh * (H // 2):
                                      ii * H + (hh + 1) * (H // 2)],
                            start=(ii == 0), stop=(ii == ISH // P128 - 1))
                    nc.vector.tensor_copy(
                        ost[:, hh * (H // 2):(hh + 1) * (H // 2)], dps[:])
                nc.gpsimd.dma_start(
                    out=so_d[cc * P128:(cc + 1) * P128, :], in_=ost[:])

    nc.compile()
    return nc


def _prepare(inputs):
    """Host-side dispatch prep: returns (in_maps, P, slot_expert, tok_lists)."""
    x = np.ascontiguousarray(inputs["hidden_states"], dtype=np.float32)
    gate_w = np.asarray(inputs["gate_w"], dtype=np.float32)
    e_bias = np.asarray(inputs["e_bias"], dtype=np.float32)
    w_gate = np.asarray(inputs["w_gate"], dtype=np.float32)
    w_up = np.asarray(inputs["w_up"], dtype=np.float32)
    w_down = np.asarray(inputs["w_down"], dtype=np.float32)
    ws_gate = np.asarray(inputs["ws_gate"], dtype=np.float32)
    ws_up = np.asarray(inputs["ws_up"], dtype=np.float32)
    ws_down = np.asarray(inputs["ws_down"], dtype=np.float32)

    # ---- dispatch metadata ----
    emask, comb = _host_routing(x, gate_w, e_bias)
    counts = emask.sum(0).astype(np.int64)
    order = np.argsort(-counts, kind="stable")
    tok_lists = [np.nonzero(emask[:, e])[0] for e in range(E)]
    tiers = [[int(order[k * NCORES + c]) for c in range(NCORES)]
             for k in range(S)]
    P = [_pad8(max(counts[e] for e in tier)) for tier in tiers]
    NCC = [(P[k] + P128 - 1) // P128 for k in range(S)]
    COL = [sum(NCC[:k]) for k in range(S)]
    BL = [HT * P128 + P[k] for k in range(S)]
    W = [HT * BL[k] + (I // P128) * H for k in range(S)]

    xb = x.astype(BF)
    wgb = w_gate.astype(BF)
    wub = w_up.astype(BF)
    wdb = w_down.astype(BF)

    # shared-expert packs (per core)
    xt = np.empty((P128, HT * T), dtype=BF)
    for h in range(HT):
        xt[:, h * T:(h + 1) * T] = xb[:, h * P128:(h + 1) * P128].T

    in_maps = []
    slot_expert = np.zeros((NCORES, S), dtype=np.int64)
    for c in range(NCORES):
        wsgu = np.empty((P128, HT * 2 * ISH), dtype=BF)
        for h in range(HT):
            o = h * 2 * ISH
            wsgu[:, o:o + ISH] = \
                ws_gate[h * P128:(h + 1) * P128, c * ISH:(c + 1) * ISH]
            wsgu[:, o + ISH:o + 2 * ISH] = \
                ws_up[h * P128:(h + 1) * P128, c * ISH:(c + 1) * ISH]
        wsd = np.empty((P128, (ISH // P128) * H), dtype=BF)
        for ii in range(ISH // P128):
            wsd[:, ii * H:(ii + 1) * H] = \
                ws_down[c * ISH + ii * P128:c * ISH + (ii + 1) * P128, :]
        m = {"xt": xt, "wsgu": wsgu, "wsd": wsd}

        scl = np.zeros((P128, sum(NCC)), dtype=np.float32)
        for k in range(S):
            e = tiers[k][c]
            slot_expert[c, k] = e
            toks = tok_lists[e]
            n = len(toks)
            mg = np.zeros((P128, W[k]), dtype=BF)
            xe = xb[toks].T                        # [H, n]
            for h in range(HT):
                o = h * BL[k]
                mg[:, o:o + I] = wgb[e, h * P128:(h + 1) * P128, :]
                mg[:, o + I:o + 2 * I] = wub[e, h * P128:(h + 1) * P128, :]
                mg[:, o + 2 * I:o + 2 * I + n] = \
                    xe[h * P128:(h + 1) * P128, :]
            wdoff = HT * BL[k]
            for ii in range(I // P128):
                mg[:, wdoff + ii * H:wdoff + (ii + 1) * H] = \
                    wdb[e, ii * P128:(ii + 1) * P128, :]
            m[f"mg{k}"] = mg
            cw = comb[toks, e]                     # [n] fp32
            for cc in range(NCC[k]):
                j0 = cc * P128
                j1 = min(n, j0 + P128)
                if j1 > j0:
                    scl[:j1 - j0, COL[k] + cc] = cw[j0:j1]
        m["scl"] = scl
        in_maps.append(m)

    return in_maps, P, slot_expert, tok_lists


def _recombine(results, slot_expert, tok_lists):
    out = np.zeros((T, H), dtype=np.float32)
    for c in range(NCORES):
        out += np.asarray(results[c]["so"], dtype=np.float32)
    for c in range(NCORES):
        for k in range(S):
            e = slot_expert[c, k]
            toks = tok_lists[e]
            ro = np.asarray(results[c][f"ro{k}"][:len(toks)], dtype=np.float32)
            out[toks] += ro
    return out


def kernel(**inputs):
    global LAST_RESULTS
    in_maps, P, slot_expert, tok_lists = _prepare(inputs)
    nc = _build_program(P)
    trace = bool(int(os.environ.get("KERNEL_TRACE", "0")))
    if trace:
        trace = _install_ntff_hook()
    LAST_RESULTS = run_bass_kernel_spmd(
        nc, in_maps, list(range(NCORES)), trace=trace)
    results = LAST_RESULTS.results
    return _recombine(results, slot_expert, tok_lists)


# revision 8
# speedup vs baseline: 1.8791x; 1.1034x over previous
"""DeepSeek-V3 MoE block on 8 trn2 NeuronCores.

Expert-parallel sparse MoE, bf16 datapath (fp32 PSUM accumulation):
  - host computes routing + combine weights in fp32 numpy (dispatch
    metadata, bit-matched to the reference's sigmoid/top-k math)
  - token load is balanced by splitting oversized experts into near-equal
    parts: the 8x5 (core x slot) grid of cells holds 40 expert-parts, slot
    capacity P[k] = that tier's max part size, so every core runs the
    identical program (SPMD); cells of a split expert re-load its weights
  - per-cell inputs (gate/up weights, gathered tokens, down weights) are
    host-packed into one [128, W] bf16 DRAM tensor, moved by a few large
    DMAs ordered so the PE streams right behind the DMA engines
  - device: per-expert gated MLP (bf16 matmuls, AF.Silu), down-projection
    computed transposed (out = [H, tokens]) so each slot's result leaves
    in a single packed DMA; combine-scaling happens in the host
    scatter-add, shared-expert MLP sharded 8-way on the intermediate dim
  - a burst of dummy matmuls at kernel start warms the PE clock (HAM)
    while the first weights stream in
"""

import os
import sys

sys.path.insert(0, "/opt/trn_rl_repo")

import numpy as np
import ml_dtypes

import concourse.bacc as bacc
import concourse.bass as bass
import concourse.mybir as mybir
import concourse.tile as tile
from concourse.bass_utils import run_bass_kernel_spmd

F32 = mybir.dt.float32
BF16 = mybir.dt.bfloat16
AF = mybir.ActivationFunctionType

BF = ml_dtypes.bfloat16

T, H, I, IS, E = 1024, 1024, 512, 2048, 32
G, TOPK_GROUP, TOP_K = 8, 4, 8
SCALE = 2.5
NCORES = 8
S = 5                    # expert-part slots per core
ISH = IS // NCORES       # shared-expert intermediate shard
P128 = 128
HT = H // P128           # 8 h-tiles
NWARM = 36               # dummy matmuls to warm the PE clock gate

LAST_RESULTS = None      # BassKernelResults of the most recent run


def _install_ntff_hook():
    """Provide antenv.axon_hooks + the ctypes NTFF profile hook when the
    container image lacks them (needed only for trace=True)."""
    import contextlib
    import ctypes
    import types

    try:
        from antenv.axon_hooks import get_axon_ntff_profile_hook  # noqa: F401
        return True
    except ImportError:
        pass
    try:
        import antenv
        so_path = "/opt/axon/libaxon_pjrt.so"
        lib = ctypes.CDLL(so_path)
        if not hasattr(lib, "axon_start_nrt_profile"):
            return False
        lib.axon_start_nrt_profile.argtypes = [
            ctypes.POINTER(ctypes.c_int64), ctypes.c_size_t]
        lib.axon_start_nrt_profile.restype = ctypes.c_int64
        lib.axon_stop_nrt_profile.argtypes = [ctypes.c_char_p]
        lib.axon_stop_nrt_profile.restype = ctypes.c_int64

        @contextlib.contextmanager
        def _hook(output_dir, device_ids):
            import jax
            jax.devices()
            if device_ids:
                ids = (ctypes.c_int64 * len(device_ids))(*device_ids)
                rc = lib.axon_start_nrt_profile(ids, len(device_ids))
            else:
                rc = lib.axon_start_nrt_profile(None, 0)
            if rc != 0:
                raise RuntimeError(f"axon_start_nrt_profile rc={rc}")
            try:
                yield
            finally:
                n = lib.axon_stop_nrt_profile(str(output_dir).encode())
                print(f"ntff profile: {n} file(s) -> {output_dir}",
                      file=sys.stderr)

        mod = types.ModuleType("antenv.axon_hooks")
        _state = {"hook": _hook}
        mod.set_axon_ntff_profile_hook = lambda h: _state.__setitem__("hook", h)
        mod.get_axon_ntff_profile_hook = lambda: _state["hook"]
        sys.modules["antenv.axon_hooks"] = mod
        antenv.axon_hooks = mod
        return True
    except Exception:
        return False


def _host_routing(x, gate_w, e_bias):
    """fp32 numpy mirror of reference._routing_combine.

    Returns (emask [T,E] bool, comb [T,E] fp32 combine weights)."""
    logits = x.astype(np.float32) @ gate_w.T.astype(np.float32)
    scores = 1.0 / (1.0 + np.exp(-logits))
    swb = scores + e_bias[None, :]
    swb_g = swb.reshape(T, G, E // G)
    gs = np.sort(swb_g, axis=-1)[..., -2:].sum(-1)          # top-2 sum per group
    thr4 = np.sort(gs, axis=-1)[:, -TOPK_GROUP][:, None]
    gmask = (gs >= thr4).astype(np.float32)
    smask = np.repeat(gmask, E // G, axis=-1)
    masked = swb * smask
    thr8 = np.sort(masked, axis=-1)[:, -TOP_K][:, None]
    emask = masked >= thr8
    sc = scores * emask
    comb = sc / (sc.sum(-1, keepdims=True) + 1e-20) * SCALE
    return emask, comb


def _pad8(n):
    return max(16, ((int(n) + 7) // 8) * 8)


def _chunks(p, limit=512):
    out = []
    o = 0
    while o < p:
        w = min(limit, p - o)
        out.append((o, w))
        o += w
    return out


def _split_cells(counts, tok_lists):
    """Balance 32 experts into NCORES*S cells by splitting big experts into
    near-equal parts. Returns cells: list of (expert, tok_array) sorted by
    descending part size, padded with (None, []) to exactly NCORES*S."""
    ncell = NCORES * S
    # smallest capacity C with sum(ceil(c/C)) <= ncell
    lo, hi = 1, int(max(counts))
    while lo < hi:
        mid = (lo + hi) // 2
        if sum(-(-int(c) // mid) for c in counts if c > 0) <= ncell:
            hi = mid
        else:
            lo = mid + 1
    C = lo
    cells = []
    for e in range(E):
        toks = tok_lists[e]
        n = len(toks)
        parts = max(1, -(-n // C))
        for j in range(parts):
            cells.append((e, toks[(j * n) // parts:((j + 1) * n) // parts]))
    cells.sort(key=lambda c: -len(c[1]))
    while len(cells) < ncell:
        cells.append((None, np.zeros((0,), dtype=np.int64)))
    return cells[:ncell]


def _build_program(P):
    """Emit the SPMD Bass program for slot capacities P (list of S ints)."""
    nc = bacc.Bacc(target_bir_lowering=False, debug=False)

    # per-slot packed input: 8 h-blocks of [wg_h(512) | wu_h(512) | xe_h(P)]
    # then 4 wd i-blocks of [128, 1024]
    BL = [HT * P128 + P[k] for k in range(S)]             # h-block width
    WDO = [HT * BL[k] for k in range(S)]                  # wd offset
    W = [WDO[k] + (I // P128) * H for k in range(S)]

    wk_d = nc.dram_tensor("wk", [P128, P128], BF16, kind="ExternalInput")
    mg_d = [nc.dram_tensor(f"mg{k}", [P128, W[k]], BF16, kind="ExternalInput")
            for k in range(S)]
    wsgu_d = nc.dram_tensor("wsgu", [P128, HT * 2 * ISH], BF16,
                            kind="ExternalInput")
    xt_d = nc.dram_tensor("xt", [P128, HT * T], BF16, kind="ExternalInput")
    wsd_d = nc.dram_tensor("wsd", [P128, (ISH // P128) * H], BF16,
                           kind="ExternalInput")
    # outputs transposed: [128, 8*P] h-major blocks (columns = tokens)
    ro_d = [nc.dram_tensor(f"ro{k}", [P128, HT * P[k]], BF16,
                           kind="ExternalOutput") for k in range(S)]
    so_d = nc.dram_tensor("so", [P128, HT * T], BF16, kind="ExternalOutput")

    NII = I // P128       # 4 expert i-tiles
    NIS = ISH // P128     # 2 shared i-tiles

    with tile.TileContext(nc) as tc:
        with (
            tc.tile_pool(name="const", bufs=1) as cpool,
            tc.tile_pool(name="acts", bufs=2) as apool,
            tc.tile_pool(name="stage", bufs=3) as stpool,
            tc.tile_pool(name="ps", bufs=8, space="PSUM") as ps,
        ):
            # ---- input DMAs, in arrival order (single HWDGE FIFO ring) ----
            wk_t = cpool.tile([P128, P128], BF16, tag="wk")
            nc.sync.dma_start(out=wk_t[:], in_=wk_d[:])

            mg_t = []
            for k in range(S):
                m = cpool.tile([P128, W[k]], BF16, tag="mg", bufs=3,
                               name=f"mg{k}")
                mg_t.append(m)
                if k == 0:
                    # fine-grained: PE streams h-block by h-block
                    for h in range(HT):
                        nc.sync.dma_start(
                            out=m[:, h * BL[k]:(h + 1) * BL[k]],
                            in_=mg_d[k][:, h * BL[k]:(h + 1) * BL[k]])
                    nc.sync.dma_start(out=m[:, WDO[k]:],
                                      in_=mg_d[k][:, WDO[k]:])
                else:
                    nc.sync.dma_start(out=m[:, :WDO[k]],
                                      in_=mg_d[k][:, :WDO[k]])
                    nc.sync.dma_start(out=m[:, WDO[k]:],
                                      in_=mg_d[k][:, WDO[k]:])
                if k == 1:
                    # shared-expert inputs land after mg0/mg1
                    wsgu_t = cpool.tile([P128, HT * 2 * ISH], BF16, tag="wsgu")
                    nc.sync.dma_start(out=wsgu_t[:], in_=wsgu_d[:])
                    xt_t = cpool.tile([P128, HT * T], BF16, tag="xt")
                    for q in range(2):
                        w2 = HT * T // 2
                        nc.sync.dma_start(out=xt_t[:, q * w2:(q + 1) * w2],
                                          in_=xt_d[:, q * w2:(q + 1) * w2])
                    wsd_t = cpool.tile([P128, NIS * H], BF16, tag="wsd")
                    nc.sync.dma_start(out=wsd_t[:], in_=wsd_d[:])

            # ---- PE clock-gate warmup: dummy matmuls, result discarded ----
            wps = ps.tile([P128, P128], F32, tag="acc", name="warm")
            for _ in range(NWARM):
                nc.tensor.matmul(wps[:], lhsT=wk_t[:], rhs=wk_t[:],
                                 start=True, stop=True)

            def gated_mlp(k):
                m = mg_t[k]
                bl, wdo, pk = BL[k], WDO[k], P[k]
                acts = [apool.tile([P128, pk], BF16, tag="acts", bufs=6,
                                   name=f"a{k}_{ii}") for ii in range(NII)]
                if k == 0:
                    # h-major: consume h-blocks as their DMAs land, using
                    # 8 PSUM banks (g0..g3, u0..u3) concurrently
                    hp = [ps.tile([P128, pk], F32, tag="acc", name=f"g{ii}")
                          for ii in range(NII)]
                    up = [ps.tile([P128, pk], F32, tag="acc", name=f"u{ii}")
                          for ii in range(NII)]
                    for h in range(HT):
                        o = h * bl
                        for ii in range(NII):
                            nc.tensor.matmul(
                                hp[ii][:],
                                lhsT=m[:, o + ii * P128:o + (ii + 1) * P128],
                                rhs=m[:, o + HT * P128:o + bl],
                                start=(h == 0), stop=(h == HT - 1))
                            nc.tensor.matmul(
                                up[ii][:],
                                lhsT=m[:, o + I + ii * P128:
                                       o + I + (ii + 1) * P128],
                                rhs=m[:, o + HT * P128:o + bl],
                                start=(h == 0), stop=(h == HT - 1))
                    for ii in range(NII):
                        sl = stpool.tile([P128, pk], F32, tag="sl", name="sl")
                        nc.scalar.activation(sl[:], hp[ii][:], AF.Silu)
                        nc.vector.tensor_mul(acts[ii][:], sl[:], up[ii][:])
                else:
                    for ii in range(NII):
                        for (mo, mw) in _chunks(pk):
                            h1 = ps.tile([P128, mw], F32, tag="acc", name="h1")
                            h2 = ps.tile([P128, mw], F32, tag="acc", name="h2")
                            for h in range(HT):
                                o = h * bl
                                nc.tensor.matmul(
                                    h1[:],
                                    lhsT=m[:, o + ii * P128:
                                           o + (ii + 1) * P128],
                                    rhs=m[:, o + HT * P128 + mo:
                                          o + HT * P128 + mo + mw],
                                    start=(h == 0), stop=(h == HT - 1))
                            for h in range(HT):
                                o = h * bl
                                nc.tensor.matmul(
                                    h2[:],
                                    lhsT=m[:, o + I + ii * P128:
                                           o + I + (ii + 1) * P128],
                                    rhs=m[:, o + HT * P128 + mo:
                                          o + HT * P128 + mo + mw],
                                    start=(h == 0), stop=(h == HT - 1))
                            sl = stpool.tile([P128, mw], F32, tag="sl",
                                             name="sl")
                            nc.scalar.activation(sl[:], h1[:], AF.Silu)
                            nc.vector.tensor_mul(acts[ii][:, mo:mo + mw],
                                                 sl[:], h2[:])

                # transposed down-proj: out block hb = [128 h, pk tokens]
                ost = stpool.tile([P128, HT * pk], BF16, tag="ost", bufs=2,
                                  name="ost")
                for hb in range(HT):
                    for (mo, mw) in _chunks(pk):
                        dps = ps.tile([P128, mw], F32, tag="acc", name="dps")
                        for ii in range(NII):
                            nc.tensor.matmul(
                                dps[:],
                                lhsT=m[:, wdo + ii * H + hb * P128:
                                       wdo + ii * H + (hb + 1) * P128],
                                rhs=acts[ii][:, mo:mo + mw],
                                start=(ii == 0), stop=(ii == NII - 1))
                        nc.scalar.activation(
                            ost[:, hb * pk + mo:hb * pk + mo + mw],
                            dps[:], AF.Copy)
                nc.scalar.dma_start(out=ro_d[k][:], in_=ost[:])

            def shared_mlp():
                acts_s = [apool.tile([P128, T], BF16, tag="acts", bufs=6,
                                     name=f"as{ii}") for ii in range(NIS)]
                for ii in range(NIS):
                    for (mo, mw) in _chunks(T):
                        h1 = ps.tile([P128, mw], F32, tag="acc", name="h1")
                        h2 = ps.tile([P128, mw], F32, tag="acc", name="h2")
                        for h in range(HT):
                            nc.tensor.matmul(
                                h1[:],
                                lhsT=wsgu_t[:, h * 2 * ISH + ii * P128:
                                            h * 2 * ISH + (ii + 1) * P128],
                                rhs=xt_t[:, h * T + mo:h * T + mo + mw],
                                start=(h == 0), stop=(h == HT - 1))
                        for h in range(HT):
                            nc.tensor.matmul(
                                h2[:],
                                lhsT=wsgu_t[:, h * 2 * ISH + ISH + ii * P128:
                                            h * 2 * ISH + ISH + (ii + 1) * P128],
                                rhs=xt_t[:, h * T + mo:h * T + mo + mw],
                                start=(h == 0), stop=(h == HT - 1))
                        sl = stpool.tile([P128, mw], F32, tag="sl", name="sl")
                        nc.scalar.activation(sl[:], h1[:], AF.Silu)
                        nc.vector.tensor_mul(acts_s[ii][:, mo:mo + mw],
                                             sl[:], h2[:])
                for half in range(2):
                    ost = stpool.tile([P128, HT * T // 2], BF16, tag="osts",
                                      bufs=2, name="osts")
                    for hb in range(HT // 2):
                        hbb = half * (HT // 2) + hb
                        for (mo, mw) in _chunks(T):
                            dps = ps.tile([P128, mw], F32, tag="acc",
                                          name="dps")
                            for ii in range(NIS):
                                nc.tensor.matmul(
                                    dps[:],
                                    lhsT=wsd_t[:, ii * H + hbb * P128:
                                               ii * H + (hbb + 1) * P128],
                                    rhs=acts_s[ii][:, mo:mo + mw],
                                    start=(ii == 0), stop=(ii == NIS - 1))
                            nc.vector.tensor_copy(
                                ost[:, hb * T + mo:hb * T + mo + mw], dps[:])
                    nc.scalar.dma_start(
                        out=so_d[:, half * (HT * T // 2):
                                 (half + 1) * (HT * T // 2)],
                        in_=ost[:])

            gated_mlp(0)
            gated_mlp(1)
            gated_mlp(2)
            shared_mlp()
            gated_mlp(3)
            gated_mlp(4)

    nc.compile()
    return nc


def _prepare(inputs):
    """Host-side dispatch prep: returns (in_maps, P, cells)."""
    x = np.ascontiguousarray(inputs["hidden_states"], dtype=np.float32)
    gate_w = np.asarray(inputs["gate_w"], dtype=np.float32)
    e_bias = np.asarray(inputs["e_bias"], dtype=np.float32)
    w_gate = np.asarray(inputs["w_gate"], dtype=np.float32)
    w_up = np.asarray(inputs["w_up"], dtype=np.float32)
    w_down = np.asarray(inputs["w_down"], dtype=np.float32)
    ws_gate = np.asarray(inputs["ws_gate"], dtype=np.float32)
    ws_up = np.asarray(inputs["ws_up"], dtype=np.float32)
    ws_down = np.asarray(inputs["ws_down"], dtype=np.float32)

    emask, comb = _host_routing(x, gate_w, e_bias)
    counts = emask.sum(0).astype(np.int64)
    tok_lists = [np.nonzero(emask[:, e])[0] for e in range(E)]
    cells = _split_cells(counts, tok_lists)     # len NCORES*S, sorted desc
    grid = [[cells[k * NCORES + c] for c in range(NCORES)] for k in range(S)]
    P = [_pad8(max(len(cell[1]) for cell in tier)) for tier in grid]
    BL = [HT * P128 + P[k] for k in range(S)]
    WDO = [HT * BL[k] for k in range(S)]
    W = [WDO[k] + (I // P128) * H for k in range(S)]

    xb = x.astype(BF)
    wgb = w_gate.astype(BF)
    wub = w_up.astype(BF)
    wdb = w_down.astype(BF)

    xt = np.empty((P128, HT * T), dtype=BF)
    for h in range(HT):
        xt[:, h * T:(h + 1) * T] = xb[:, h * P128:(h + 1) * P128].T
    wk = np.zeros((P128, P128), dtype=BF)

    in_maps = []
    for c in range(NCORES):
        wsgu = np.empty((P128, HT * 2 * ISH), dtype=BF)
        for h in range(HT):
            o = h * 2 * ISH
            wsgu[:, o:o + ISH] = \
                ws_gate[h * P128:(h + 1) * P128, c * ISH:(c + 1) * ISH]
            wsgu[:, o + ISH:o + 2 * ISH] = \
                ws_up[h * P128:(h + 1) * P128, c * ISH:(c + 1) * ISH]
        wsd = np.empty((P128, (ISH // P128) * H), dtype=BF)
        for ii in range(ISH // P128):
            wsd[:, ii * H:(ii + 1) * H] = \
                ws_down[c * ISH + ii * P128:c * ISH + (ii + 1) * P128, :]
        m = {"wk": wk, "xt": xt, "wsgu": wsgu, "wsd": wsd}

        for k in range(S):
            e, toks = grid[k][c]
            n = len(toks)
            mg = np.zeros((P128, W[k]), dtype=BF)
            if e is not None:
                xe = xb[toks].T                    # [H, n]
                for h in range(HT):
                    o = h * BL[k]
                    mg[:, o:o + I] = wgb[e, h * P128:(h + 1) * P128, :]
                    mg[:, o + I:o + 2 * I] = wub[e, h * P128:(h + 1) * P128, :]
                    if n:
                        mg[:, o + 2 * I:o + 2 * I + n] = \
                            xe[h * P128:(h + 1) * P128, :]
                for ii in range(I // P128):
                    mg[:, WDO[k] + ii * H:WDO[k] + (ii + 1) * H] = \
                        wdb[e, ii * P128:(ii + 1) * P128, :]
            m[f"mg{k}"] = mg
        in_maps.append(m)

    return in_maps, P, grid, comb


def _recombine(results, P, grid, comb):
    out = np.zeros((T, H), dtype=np.float32)
    # shared partials: so[p, h*T + t] = partial[t, h*128+p]
    for c in range(NCORES):
        so = np.asarray(results[c]["so"], dtype=np.float32)
        out += so.reshape(P128, HT, T).transpose(2, 1, 0).reshape(T, H)
    # routed: ro[p, hb*P + j] = down_out[token j, hb*128+p]; scale on host
    for c in range(NCORES):
        for k in range(S):
            e, toks = grid[k][c]
            n = len(toks)
            if e is None or n == 0:
                continue
            ro = np.asarray(results[c][f"ro{k}"], dtype=np.float32)
            contrib = ro.reshape(P128, HT, P[k])[:, :, :n]   # [128, HT, n]
            contrib = contrib.transpose(2, 1, 0).reshape(n, H)
            out[toks] += contrib * comb[toks, e][:, None]
    return out


def kernel(**inputs):
    global LAST_RESULTS
    in_maps, P, grid, comb = _prepare(inputs)
    nc = _build_program(P)
    trace = bool(int(os.environ.get("KERNEL_TRACE", "0")))
    if trace:
        trace = _install_ntff_hook()
    LAST_RESULTS = run_bass_kernel_spmd(
        nc, in_maps, list(range(NCORES)), trace=trace)
    results = LAST_RESULTS.results
    return _recombine(results, P, grid, comb)


# revision 12
# speedup vs baseline: 1.9862x; 1.0570x over previous
"""DeepSeek-V3 MoE block on 8 trn2 NeuronCores.

Expert-parallel sparse MoE, bf16 datapath (fp32 PSUM accumulation):
  - host computes routing + combine weights in fp32 numpy (dispatch
    metadata, bit-matched to the reference's sigmoid/top-k math)
  - token load is balanced by splitting oversized experts into near-equal
    parts: the 8x5 (core x slot) grid of cells holds 40 expert-parts, slot
    capacity P[k] = that tier's max part size, so every core runs the
    identical program (SPMD); cells of a split expert re-load its weights
  - per-cell inputs (gate/up weights, gathered tokens, down weights) are
    host-packed into one [128, W] bf16 DRAM tensor, moved by a few large
    DMAs ordered so the PE streams right behind the DMA engines
  - device: per-expert gated MLP (bf16 matmuls, AF.Silu), down-projection
    computed transposed (out = [H, tokens]) so each slot's result leaves
    in a single packed DMA; combine-scaling happens in the host
    scatter-add, shared-expert MLP sharded 8-way on the intermediate dim
  - a burst of dummy matmuls at kernel start warms the PE clock (HAM)
    while the first weights stream in
"""

import os
import sys

sys.path.insert(0, "/opt/trn_rl_repo")

import numpy as np
import ml_dtypes

import concourse.bacc as bacc
import concourse.bass as bass
import concourse.mybir as mybir
import concourse.tile as tile
from concourse.bass_utils import run_bass_kernel_spmd

F32 = mybir.dt.float32
BF16 = mybir.dt.bfloat16
AF = mybir.ActivationFunctionType

BF = ml_dtypes.bfloat16

T, H, I, IS, E = 1024, 1024, 512, 2048, 32
G, TOPK_GROUP, TOP_K = 8, 4, 8
SCALE = 2.5
NCORES = 8
S = 5                    # expert-part slots per core
ISH = IS // NCORES       # shared-expert intermediate shard
P128 = 128
HT = H // P128           # 8 h-tiles
NWARM = 36               # dummy matmuls to warm the PE clock gate

LAST_RESULTS = None      # BassKernelResults of the most recent run


def _install_ntff_hook():
    """Provide antenv.axon_hooks + the ctypes NTFF profile hook when the
    container image lacks them (needed only for trace=True)."""
    import contextlib
    import ctypes
    import types

    try:
        from antenv.axon_hooks import get_axon_ntff_profile_hook  # noqa: F401
        return True
    except ImportError:
        pass
    try:
        import antenv
        so_path = "/opt/axon/libaxon_pjrt.so"
        lib = ctypes.CDLL(so_path)
        if not hasattr(lib, "axon_start_nrt_profile"):
            return False
        lib.axon_start_nrt_profile.argtypes = [
            ctypes.POINTER(ctypes.c_int64), ctypes.c_size_t]
        lib.axon_start_nrt_profile.restype = ctypes.c_int64
        lib.axon_stop_nrt_profile.argtypes = [ctypes.c_char_p]
        lib.axon_stop_nrt_profile.restype = ctypes.c_int64

        @contextlib.contextmanager
        def _hook(output_dir, device_ids):
            import jax
            jax.devices()
            if device_ids:
                ids = (ctypes.c_int64 * len(device_ids))(*device_ids)
                rc = lib.axon_start_nrt_profile(ids, len(device_ids))
            else:
                rc = lib.axon_start_nrt_profile(None, 0)
            if rc != 0:
                raise RuntimeError(f"axon_start_nrt_profile rc={rc}")
            try:
                yield
            finally:
                n = lib.axon_stop_nrt_profile(str(output_dir).encode())
                print(f"ntff profile: {n} file(s) -> {output_dir}",
                      file=sys.stderr)

        mod = types.ModuleType("antenv.axon_hooks")
        _state = {"hook": _hook}
        mod.set_axon_ntff_profile_hook = lambda h: _state.__setitem__("hook", h)
        mod.get_axon_ntff_profile_hook = lambda: _state["hook"]
        sys.modules["antenv.axon_hooks"] = mod
        antenv.axon_hooks = mod
        return True
    except Exception:
        return False


def _host_routing(x, gate_w, e_bias):
    """fp32 numpy mirror of reference._routing_combine.

    Returns (emask [T,E] bool, comb [T,E] fp32 combine weights)."""
    logits = x.astype(np.float32) @ gate_w.T.astype(np.float32)
    scores = 1.0 / (1.0 + np.exp(-logits))
    swb = scores + e_bias[None, :]
    swb_g = swb.reshape(T, G, E // G)
    gs = np.sort(swb_g, axis=-1)[..., -2:].sum(-1)          # top-2 sum per group
    thr4 = np.sort(gs, axis=-1)[:, -TOPK_GROUP][:, None]
    gmask = (gs >= thr4).astype(np.float32)
    smask = np.repeat(gmask, E // G, axis=-1)
    masked = swb * smask
    thr8 = np.sort(masked, axis=-1)[:, -TOP_K][:, None]
    emask = masked >= thr8
    sc = scores * emask
    comb = sc / (sc.sum(-1, keepdims=True) + 1e-20) * SCALE
    return emask, comb


def _pad8(n):
    return max(16, ((int(n) + 7) // 8) * 8)


def _chunks(p, limit=512):
    out = []
    o = 0
    while o < p:
        w = min(limit, p - o)
        out.append((o, w))
        o += w
    return out


def _split_cells(counts, tok_lists):
    """Balance 32 experts into NCORES*S cells by splitting big experts into
    near-equal parts. Returns cells: list of (expert, tok_array) sorted by
    descending part size, padded with (None, []) to exactly NCORES*S."""
    ncell = NCORES * S
    # smallest capacity C with sum(ceil(c/C)) <= ncell
    lo, hi = 1, int(max(counts))
    while lo < hi:
        mid = (lo + hi) // 2
        if sum(-(-int(c) // mid) for c in counts if c > 0) <= ncell:
            hi = mid
        else:
            lo = mid + 1
    C = lo
    cells = []
    for e in range(E):
        toks = tok_lists[e]
        n = len(toks)
        parts = max(1, -(-n // C))
        for j in range(parts):
            cells.append((e, toks[(j * n) // parts:((j + 1) * n) // parts]))
    cells.sort(key=lambda c: -len(c[1]))
    while len(cells) < ncell:
        cells.append((None, np.zeros((0,), dtype=np.int64)))
    return cells[:ncell]


def _build_program(P):
    """Emit the SPMD Bass program for slot capacities P (list of S ints)."""
    nc = bacc.Bacc(target_bir_lowering=False, debug=False)

    # per-slot inputs: gu = 8 h-blocks of [wg_h(512) | wu_h(512)];
    # xe = 8 h-blocks of [128, P]; wd = 4 i-blocks of [128, 1024]
    gu_d = [nc.dram_tensor(f"gu{k}", [P128, HT * 2 * I], BF16,
                           kind="ExternalInput") for k in range(S)]
    xe_d = [nc.dram_tensor(f"xe{k}", [P128, HT * P[k]], BF16,
                           kind="ExternalInput") for k in range(S)]
    wd_d = [nc.dram_tensor(f"wd{k}", [P128, (I // P128) * H], BF16,
                           kind="ExternalInput") for k in range(S)]
    wk_d = nc.dram_tensor("wk", [P128, P128], BF16, kind="ExternalInput")
    wsgu_d = nc.dram_tensor("wsgu", [P128, HT * 2 * ISH], BF16,
                            kind="ExternalInput")
    xt_d = nc.dram_tensor("xt", [P128, HT * T], BF16, kind="ExternalInput")
    wsd_d = nc.dram_tensor("wsd", [P128, (ISH // P128) * H], BF16,
                           kind="ExternalInput")
    # outputs transposed: [128, 8*P] h-major blocks (columns = tokens)
    ro_d = [nc.dram_tensor(f"ro{k}", [P128, HT * P[k]], BF16,
                           kind="ExternalOutput") for k in range(S)]
    so_d = nc.dram_tensor("so", [P128, HT * T], BF16, kind="ExternalOutput")

    NII = I // P128       # 4 expert i-tiles
    NIS = ISH // P128     # 2 shared i-tiles

    with tile.TileContext(nc) as tc:
        with (
            tc.tile_pool(name="const", bufs=1) as cpool,
            tc.tile_pool(name="acts", bufs=2) as apool,
            tc.tile_pool(name="stage", bufs=3) as stpool,
            tc.tile_pool(name="ps", bufs=8, space="PSUM") as ps,
        ):
            # ---- input DMAs, in consumption order (HWDGE FIFO ring).
            # Uniform ~0.25-1MB chunks keep per-chunk completion latency low
            # so consumers never wait on a half-delivered multi-MB block.
            wk_t = cpool.tile([P128, P128], BF16, tag="wk")
            nc.sync.dma_start(out=wk_t[:], in_=wk_d[:])

            gu_t, xe_t, wd_t = [], [], []

            def slot_dmas(k, fine):
                g = cpool.tile([P128, HT * 2 * I], BF16, tag="gu", bufs=3,
                               name=f"gu{k}")
                x = cpool.tile([P128, HT * P[k]], BF16, tag="xe", bufs=3,
                               name=f"xe{k}")
                w = cpool.tile([P128, (I // P128) * H], BF16, tag="wd",
                               bufs=3, name=f"wd{k}")
                gu_t.append(g); xe_t.append(x); wd_t.append(w)
                if fine:
                    hw = HT * P[k] // 2
                    nc.sync.dma_start(out=x[:, :hw], in_=xe_d[k][:, :hw])
                    for h in range(HT):
                        nc.sync.dma_start(
                            out=g[:, h * 2 * I:(h + 1) * 2 * I],
                            in_=gu_d[k][:, h * 2 * I:(h + 1) * 2 * I])
                        if h == 3:
                            nc.sync.dma_start(out=x[:, hw:],
                                              in_=xe_d[k][:, hw:])
                else:
                    nc.sync.dma_start(out=g[:, :HT * I],
                                      in_=gu_d[k][:, :HT * I])
                    nc.sync.dma_start(out=g[:, HT * I:],
                                      in_=gu_d[k][:, HT * I:])
                    nc.sync.dma_start(out=x[:], in_=xe_d[k][:])
                nc.sync.dma_start(out=w[:], in_=wd_d[k][:])

            slot_dmas(0, True)
            slot_dmas(1, False)
            # shared-expert inputs land after gu0/gu1
            wsgu_t = cpool.tile([P128, HT * 2 * ISH], BF16, tag="wsgu")
            nc.sync.dma_start(out=wsgu_t[:], in_=wsgu_d[:])
            xt_t = cpool.tile([P128, HT * T], BF16, tag="xt")
            for q in range(2):
                w2 = HT * T // 2
                nc.sync.dma_start(out=xt_t[:, q * w2:(q + 1) * w2],
                                  in_=xt_d[:, q * w2:(q + 1) * w2])
            wsd_t = cpool.tile([P128, NIS * H], BF16, tag="wsd")
            nc.sync.dma_start(out=wsd_t[:], in_=wsd_d[:])
            for k in range(2, S):
                slot_dmas(k, False)

            # ---- PE clock-gate warmup: dummy matmuls, result discarded ----
            wps = ps.tile([P128, P128], F32, tag="acc", name="warm")
            for _ in range(NWARM):
                nc.tensor.matmul(wps[:], lhsT=wk_t[:], rhs=wk_t[:],
                                 start=True, stop=True)

            def gated_mlp(k):
                g, x, w = gu_t[k], xe_t[k], wd_t[k]
                pk = P[k]
                acts = [apool.tile([P128, pk], BF16, tag="acts", bufs=6,
                                   name=f"a{ii}") for ii in range(NII)]
                if k == 0:
                    # h-major: consume h-blocks as their DMAs land, using
                    # 8 PSUM banks (g0..g3, u0..u3) concurrently
                    hp = [ps.tile([P128, pk], F32, tag="acc", name=f"g{ii}")
                          for ii in range(NII)]
                    up = [ps.tile([P128, pk], F32, tag="acc", name=f"u{ii}")
                          for ii in range(NII)]
                    for h in range(HT):
                        o = h * 2 * I
                        for ii in range(NII):
                            nc.tensor.matmul(
                                hp[ii][:],
                                lhsT=g[:, o + ii * P128:o + (ii + 1) * P128],
                                rhs=x[:, h * pk:(h + 1) * pk],
                                start=(h == 0), stop=(h == HT - 1))
                            nc.tensor.matmul(
                                up[ii][:],
                                lhsT=g[:, o + I + ii * P128:
                                       o + I + (ii + 1) * P128],
                                rhs=x[:, h * pk:(h + 1) * pk],
                                start=(h == 0), stop=(h == HT - 1))
                    for ii in range(NII):
                        sl = stpool.tile([P128, pk], F32, tag="sl", name="sl")
                        nc.scalar.activation(sl[:], hp[ii][:], AF.Silu)
                        nc.vector.tensor_mul(acts[ii][:], sl[:], up[ii][:])
                else:
                    for ii in range(NII):
                        for (mo, mw) in _chunks(pk):
                            h1 = ps.tile([P128, mw], F32, tag="acc", name="h1")
                            h2 = ps.tile([P128, mw], F32, tag="acc", name="h2")
                            for h in range(HT):
                                o = h * 2 * I
                                nc.tensor.matmul(
                                    h1[:],
                                    lhsT=g[:, o + ii * P128:
                                           o + (ii + 1) * P128],
                                    rhs=x[:, h * pk + mo:h * pk + mo + mw],
                                    start=(h == 0), stop=(h == HT - 1))
                            for h in range(HT):
                                o = h * 2 * I
                                nc.tensor.matmul(
                                    h2[:],
                                    lhsT=g[:, o + I + ii * P128:
                                           o + I + (ii + 1) * P128],
                                    rhs=x[:, h * pk + mo:h * pk + mo + mw],
                                    start=(h == 0), stop=(h == HT - 1))
                            sl = stpool.tile([P128, mw], F32, tag="sl",
                                             name="sl")
                            nc.scalar.activation(sl[:], h1[:], AF.Silu)
                            nc.vector.tensor_mul(acts[ii][:, mo:mo + mw],
                                                 sl[:], h2[:])

                # transposed down-proj: out block hb = [128 h, pk tokens]
                ost = stpool.tile([P128, HT * pk], BF16, tag="ost", bufs=2,
                                  name="ost")
                for hb in range(HT):
                    for (mo, mw) in _chunks(pk):
                        dps = ps.tile([P128, mw], F32, tag="acc", name="dps")
                        for ii in range(NII):
                            nc.tensor.matmul(
                                dps[:],
                                lhsT=w[:, ii * H + hb * P128:
                                       ii * H + (hb + 1) * P128],
                                rhs=acts[ii][:, mo:mo + mw],
                                start=(ii == 0), stop=(ii == NII - 1))
                        nc.scalar.activation(
                            ost[:, hb * pk + mo:hb * pk + mo + mw],
                            dps[:], AF.Copy)
                nc.scalar.dma_start(out=ro_d[k][:], in_=ost[:])

            def shared_mlp():
                acts_s = [apool.tile([P128, T], BF16, tag="acts", bufs=6,
                                     name=f"as{ii}") for ii in range(NIS)]
                for ii in range(NIS):
                    for (mo, mw) in _chunks(T):
                        h1 = ps.tile([P128, mw], F32, tag="acc", name="h1")
                        h2 = ps.tile([P128, mw], F32, tag="acc", name="h2")
                        for h in range(HT):
                            nc.tensor.matmul(
                                h1[:],
                                lhsT=wsgu_t[:, h * 2 * ISH + ii * P128:
                                            h * 2 * ISH + (ii + 1) * P128],
                                rhs=xt_t[:, h * T + mo:h * T + mo + mw],
                                start=(h == 0), stop=(h == HT - 1))
                        for h in range(HT):
                            nc.tensor.matmul(
                                h2[:],
                                lhsT=wsgu_t[:, h * 2 * ISH + ISH + ii * P128:
                                            h * 2 * ISH + ISH + (ii + 1) * P128],
                                rhs=xt_t[:, h * T + mo:h * T + mo + mw],
                                start=(h == 0), stop=(h == HT - 1))
                        sl = stpool.tile([P128, mw], F32, tag="sl", name="sl")
                        nc.scalar.activation(sl[:], h1[:], AF.Silu)
                        nc.vector.tensor_mul(acts_s[ii][:, mo:mo + mw],
                                             sl[:], h2[:])
                for half in range(2):
                    ost = stpool.tile([P128, HT * T // 2], BF16, tag="osts",
                                      bufs=2, name="osts")
                    for hb in range(HT // 2):
                        hbb = half * (HT // 2) + hb
                        for (mo, mw) in _chunks(T):
                            dps = ps.tile([P128, mw], F32, tag="acc",
                                          name="dps")
                            for ii in range(NIS):
                                nc.tensor.matmul(
                                    dps[:],
                                    lhsT=wsd_t[:, ii * H + hbb * P128:
                                               ii * H + (hbb + 1) * P128],
                                    rhs=acts_s[ii][:, mo:mo + mw],
                                    start=(ii == 0), stop=(ii == NIS - 1))
                            nc.vector.tensor_copy(
                                ost[:, hb * T + mo:hb * T + mo + mw], dps[:])
                    nc.scalar.dma_start(
                        out=so_d[:, half * (HT * T // 2):
                                 (half + 1) * (HT * T // 2)],
                        in_=ost[:])

            gated_mlp(0)
            gated_mlp(1)
            shared_mlp()
            gated_mlp(2)
            gated_mlp(3)
            gated_mlp(4)

    nc.compile()
    return nc


def _prepare(inputs):
    """Host-side dispatch prep: returns (in_maps, P, cells)."""
    x = np.ascontiguousarray(inputs["hidden_states"], dtype=np.float32)
    gate_w = np.asarray(inputs["gate_w"], dtype=np.float32)
    e_bias = np.asarray(inputs["e_bias"], dtype=np.float32)
    w_gate = np.asarray(inputs["w_gate"], dtype=np.float32)
    w_up = np.asarray(inputs["w_up"], dtype=np.float32)
    w_down = np.asarray(inputs["w_down"], dtype=np.float32)
    ws_gate = np.asarray(inputs["ws_gate"], dtype=np.float32)
    ws_up = np.asarray(inputs["ws_up"], dtype=np.float32)
    ws_down = np.asarray(inputs["ws_down"], dtype=np.float32)

    emask, comb = _host_routing(x, gate_w, e_bias)
    counts = emask.sum(0).astype(np.int64)
    tok_lists = [np.nonzero(emask[:, e])[0] for e in range(E)]
    cells = _split_cells(counts, tok_lists)     # len NCORES*S, sorted desc
    grid = [[cells[k * NCORES + c] for c in range(NCORES)] for k in range(S)]
    P = [_pad8(max(len(cell[1]) for cell in tier)) for tier in grid]

    xb = x.astype(BF)
    wgb = w_gate.astype(BF)
    wub = w_up.astype(BF)
    wdb = w_down.astype(BF)

    xt = np.empty((P128, HT * T), dtype=BF)
    for h in range(HT):
        xt[:, h * T:(h + 1) * T] = xb[:, h * P128:(h + 1) * P128].T
    wk = np.zeros((P128, P128), dtype=BF)

    in_maps = []
    for c in range(NCORES):
        wsgu = np.empty((P128, HT * 2 * ISH), dtype=BF)
        for h in range(HT):
            o = h * 2 * ISH
            wsgu[:, o:o + ISH] = \
                ws_gate[h * P128:(h + 1) * P128, c * ISH:(c + 1) * ISH]
            wsgu[:, o + ISH:o + 2 * ISH] = \
                ws_up[h * P128:(h + 1) * P128, c * ISH:(c + 1) * ISH]
        wsd = np.empty((P128, (ISH // P128) * H), dtype=BF)
        for ii in range(ISH // P128):
            wsd[:, ii * H:(ii + 1) * H] = \
                ws_down[c * ISH + ii * P128:c * ISH + (ii + 1) * P128, :]
        m = {"wk": wk, "xt": xt, "wsgu": wsgu, "wsd": wsd}

        for k in range(S):
            e, toks = grid[k][c]
            n = len(toks)
            gu = np.zeros((P128, HT * 2 * I), dtype=BF)
            xp = np.zeros((P128, HT * P[k]), dtype=BF)
            wd = np.zeros((P128, (I // P128) * H), dtype=BF)
            if e is not None:
                xe = xb[toks].T                    # [H, n]
                for h in range(HT):
                    o = h * 2 * I
                    gu[:, o:o + I] = wgb[e, h * P128:(h + 1) * P128, :]
                    gu[:, o + I:o + 2 * I] = wub[e, h * P128:(h + 1) * P128, :]
                    if n:
                        xp[:, h * P[k]:h * P[k] + n] = \
                            xe[h * P128:(h + 1) * P128, :]
                for ii in range(I // P128):
                    wd[:, ii * H:(ii + 1) * H] = \
                        wdb[e, ii * P128:(ii + 1) * P128, :]
            m[f"gu{k}"] = gu
            m[f"xe{k}"] = xp
            m[f"wd{k}"] = wd
        in_maps.append(m)

    return in_maps, P, grid, comb


def _recombine(results, P, grid, comb):
    out = np.zeros((T, H), dtype=np.float32)
    # shared partials: so[p, h*T + t] = partial[t, h*128+p]
    for c in range(NCORES):
        so = np.asarray(results[c]["so"], dtype=np.float32)
        out += so.reshape(P128, HT, T).transpose(2, 1, 0).reshape(T, H)
    # routed: ro[p, hb*P + j] = down_out[token j, hb*128+p]; scale on host
    for c in range(NCORES):
        for k in range(S):
            e, toks = grid[k][c]
            n = len(toks)
            if e is None or n == 0:
                continue
            ro = np.asarray(results[c][f"ro{k}"], dtype=np.float32)
            contrib = ro.reshape(P128, HT, P[k])[:, :, :n]   # [128, HT, n]
            contrib = contrib.transpose(2, 1, 0).reshape(n, H)
            out[toks] += contrib * comb[toks, e][:, None]
    return out


def kernel(**inputs):
    global LAST_RESULTS
    in_maps, P, grid, comb = _prepare(inputs)
    nc = _build_program(P)
    trace = bool(int(os.environ.get("KERNEL_TRACE", "0")))
    if trace:
        trace = _install_ntff_hook()
    LAST_RESULTS = run_bass_kernel_spmd(
        nc, in_maps, list(range(NCORES)), trace=trace)
    results = LAST_RESULTS.results
    return _recombine(results, P, grid, comb)


# revision 14
# speedup vs baseline: 2.0203x; 1.0172x over previous
"""DeepSeek-V3 MoE block on 8 trn2 NeuronCores.

Expert-parallel sparse MoE, bf16 datapath (fp32 PSUM accumulation):
  - host computes routing + combine weights in fp32 numpy (dispatch
    metadata, bit-matched to the reference's sigmoid/top-k math)
  - token load is balanced by splitting oversized experts into near-equal
    parts: the 8x5 (core x slot) grid of cells holds 40 expert-parts, slot
    capacity P[k] = that tier's max part size, so every core runs the
    identical program (SPMD); cells of a split expert re-load its weights
  - per-cell inputs (gate/up weights, gathered tokens, down weights) are
    host-packed into one [128, W] bf16 DRAM tensor, moved by a few large
    DMAs ordered so the PE streams right behind the DMA engines
  - device: per-expert gated MLP (bf16 matmuls, AF.Silu), down-projection
    computed transposed (out = [H, tokens]) so each slot's result leaves
    in a single packed DMA; combine-scaling happens in the host
    scatter-add, shared-expert MLP sharded 8-way on the intermediate dim
  - a burst of dummy matmuls at kernel start warms the PE clock (HAM)
    while the first weights stream in
"""

import os
import sys

sys.path.insert(0, "/opt/trn_rl_repo")

import numpy as np
import ml_dtypes

import concourse.bacc as bacc
import concourse.bass as bass
import concourse.mybir as mybir
import concourse.tile as tile
from concourse.bass_utils import run_bass_kernel_spmd

F32 = mybir.dt.float32
BF16 = mybir.dt.bfloat16
AF = mybir.ActivationFunctionType

BF = ml_dtypes.bfloat16

T, H, I, IS, E = 1024, 1024, 512, 2048, 32
G, TOPK_GROUP, TOP_K = 8, 4, 8
SCALE = 2.5
NCORES = 8
S = 5                    # expert-part slots per core
ISH = IS // NCORES       # shared-expert intermediate shard
P128 = 128
HT = H // P128           # 8 h-tiles
NWARM = 36               # dummy matmuls to warm the PE clock gate

LAST_RESULTS = None      # BassKernelResults of the most recent run


def _install_ntff_hook():
    """Provide antenv.axon_hooks + the ctypes NTFF profile hook when the
    container image lacks them (needed only for trace=True)."""
    import contextlib
    import ctypes
    import types

    try:
        from antenv.axon_hooks import get_axon_ntff_profile_hook  # noqa: F401
        return True
    except ImportError:
        pass
    try:
        import antenv
        so_path = "/opt/axon/libaxon_pjrt.so"
        lib = ctypes.CDLL(so_path)
        if not hasattr(lib, "axon_start_nrt_profile"):
            return False
        lib.axon_start_nrt_profile.argtypes = [
            ctypes.POINTER(ctypes.c_int64), ctypes.c_size_t]
        lib.axon_start_nrt_profile.restype = ctypes.c_int64
        lib.axon_stop_nrt_profile.argtypes = [ctypes.c_char_p]
        lib.axon_stop_nrt_profile.restype = ctypes.c_int64

        @contextlib.contextmanager
        def _hook(output_dir, device_ids):
            import jax
            jax.devices()
            if device_ids:
                ids = (ctypes.c_int64 * len(device_ids))(*device_ids)
                rc = lib.axon_start_nrt_profile(ids, len(device_ids))
            else:
                rc = lib.axon_start_nrt_profile(None, 0)
            if rc != 0:
                raise RuntimeError(f"axon_start_nrt_profile rc={rc}")
            try:
                yield
            finally:
                n = lib.axon_stop_nrt_profile(str(output_dir).encode())
                print(f"ntff profile: {n} file(s) -> {output_dir}",
                      file=sys.stderr)

        mod = types.ModuleType("antenv.axon_hooks")
        _state = {"hook": _hook}
        mod.set_axon_ntff_profile_hook = lambda h: _state.__setitem__("hook", h)
        mod.get_axon_ntff_profile_hook = lambda: _state["hook"]
        sys.modules["antenv.axon_hooks"] = mod
        antenv.axon_hooks = mod
        return True
    except Exception:
        return False


def _host_routing(x, gate_w, e_bias):
    """fp32 numpy mirror of reference._routing_combine.

    Returns (emask [T,E] bool, comb [T,E] fp32 combine weights)."""
    logits = x.astype(np.float32) @ gate_w.T.astype(np.float32)
    scores = 1.0 / (1.0 + np.exp(-logits))
    swb = scores + e_bias[None, :]
    swb_g = swb.reshape(T, G, E // G)
    gs = np.sort(swb_g, axis=-1)[..., -2:].sum(-1)          # top-2 sum per group
    thr4 = np.sort(gs, axis=-1)[:, -TOPK_GROUP][:, None]
    gmask = (gs >= thr4).astype(np.float32)
    smask = np.repeat(gmask, E // G, axis=-1)
    masked = swb * smask
    thr8 = np.sort(masked, axis=-1)[:, -TOP_K][:, None]
    emask = masked >= thr8
    sc = scores * emask
    comb = sc / (sc.sum(-1, keepdims=True) + 1e-20) * SCALE
    return emask, comb


def _pad8(n):
    return max(16, ((int(n) + 7) // 8) * 8)


def _chunks(p, limit=512):
    out = []
    o = 0
    while o < p:
        w = min(limit, p - o)
        out.append((o, w))
        o += w
    return out


def _split_cells(counts, tok_lists):
    """Balance 32 experts into NCORES*S cells by splitting big experts into
    near-equal parts. Returns cells: list of (expert, tok_array) sorted by
    descending part size, padded with (None, []) to exactly NCORES*S."""
    ncell = NCORES * S
    # smallest capacity C with sum(ceil(c/C)) <= ncell
    lo, hi = 1, int(max(counts))
    while lo < hi:
        mid = (lo + hi) // 2
        if sum(-(-int(c) // mid) for c in counts if c > 0) <= ncell:
            hi = mid
        else:
            lo = mid + 1
    C = lo
    cells = []
    for e in range(E):
        toks = tok_lists[e]
        n = len(toks)
        parts = max(1, -(-n // C))
        for j in range(parts):
            cells.append((e, toks[(j * n) // parts:((j + 1) * n) // parts]))
    cells.sort(key=lambda c: -len(c[1]))
    while len(cells) < ncell:
        cells.append((None, np.zeros((0,), dtype=np.int64)))
    return cells[:ncell]


def _build_program(P):
    """Emit the SPMD Bass program for slot capacities P (list of S ints)."""
    nc = bacc.Bacc(target_bir_lowering=False, debug=False)

    # per-slot inputs: gu = 8 h-blocks of [wg_h(512) | wu_h(512)];
    # xe = 8 h-blocks of [128, P]; wd = 4 i-blocks of [128, 1024]
    gu_d = [nc.dram_tensor(f"gu{k}", [P128, HT * 2 * I], BF16,
                           kind="ExternalInput") for k in range(S)]
    xe_d = [nc.dram_tensor(f"xe{k}", [P128, HT * P[k]], BF16,
                           kind="ExternalInput") for k in range(S)]
    wd_d = [nc.dram_tensor(f"wd{k}", [P128, (I // P128) * H], BF16,
                           kind="ExternalInput") for k in range(S)]
    wk_d = nc.dram_tensor("wk", [P128, P128], BF16, kind="ExternalInput")
    wsgu_d = nc.dram_tensor("wsgu", [P128, HT * 2 * ISH], BF16,
                            kind="ExternalInput")
    xt_d = nc.dram_tensor("xt", [P128, HT * T], BF16, kind="ExternalInput")
    wsd_d = nc.dram_tensor("wsd", [P128, (ISH // P128) * H], BF16,
                           kind="ExternalInput")
    # outputs transposed: [128, 8*P] h-major blocks (columns = tokens)
    ro_d = [nc.dram_tensor(f"ro{k}", [P128, HT * P[k]], BF16,
                           kind="ExternalOutput") for k in range(S)]
    so_d = nc.dram_tensor("so", [P128, HT * T], BF16, kind="ExternalOutput")

    NII = I // P128       # 4 expert i-tiles
    NIS = ISH // P128     # 2 shared i-tiles

    with tile.TileContext(nc) as tc:
        with (
            tc.tile_pool(name="const", bufs=1) as cpool,
            tc.tile_pool(name="acts", bufs=2) as apool,
            tc.tile_pool(name="stage", bufs=3) as stpool,
            tc.tile_pool(name="ps", bufs=8, space="PSUM") as ps,
        ):
            # ---- input DMAs, in consumption order (HWDGE FIFO ring).
            # Uniform ~0.25-1MB chunks keep per-chunk completion latency low
            # so consumers never wait on a half-delivered multi-MB block.
            wk_t = cpool.tile([P128, P128], BF16, tag="wk")
            nc.sync.dma_start(out=wk_t[:], in_=wk_d[:])

            # shared-expert gate/up inputs first: best compute-per-byte,
            # consumable per xt h-block while the slot weights stream in
            wsgu_t = cpool.tile([P128, HT * 2 * ISH], BF16, tag="wsgu")
            half = HT * ISH
            nc.sync.dma_start(out=wsgu_t[:, :half], in_=wsgu_d[:, :half])
            nc.sync.dma_start(out=wsgu_t[:, half:], in_=wsgu_d[:, half:])
            xt_t = cpool.tile([P128, HT * T], BF16, tag="xt")
            for q in range(4):
                w4 = HT * T // 4
                nc.sync.dma_start(out=xt_t[:, q * w4:(q + 1) * w4],
                                  in_=xt_d[:, q * w4:(q + 1) * w4])

            gu_t, xe_t, wd_t = [], [], []

            def slot_dmas(k):
                g = cpool.tile([P128, HT * 2 * I], BF16, tag="gu", bufs=3,
                               name=f"gu{k}")
                x = cpool.tile([P128, HT * P[k]], BF16, tag="xe", bufs=3,
                               name=f"xe{k}")
                w = cpool.tile([P128, (I // P128) * H], BF16, tag="wd",
                               bufs=3, name=f"wd{k}")
                gu_t.append(g); xe_t.append(x); wd_t.append(w)
                nc.sync.dma_start(out=g[:, :HT * I], in_=gu_d[k][:, :HT * I])
                nc.sync.dma_start(out=g[:, HT * I:], in_=gu_d[k][:, HT * I:])
                nc.sync.dma_start(out=x[:], in_=xe_d[k][:])
                nc.sync.dma_start(out=w[:], in_=wd_d[k][:])

            slot_dmas(0)
            slot_dmas(1)
            slot_dmas(2)
            wsd_t = cpool.tile([P128, NIS * H], BF16, tag="wsd")
            nc.sync.dma_start(out=wsd_t[:], in_=wsd_d[:])
            slot_dmas(3)
            slot_dmas(4)

            # ---- PE clock-gate warmup: dummy matmuls, result discarded ----
            wps = ps.tile([P128, P128], F32, tag="acc", name="warm")
            for _ in range(NWARM):
                nc.tensor.matmul(wps[:], lhsT=wk_t[:], rhs=wk_t[:],
                                 start=True, stop=True)

            def gated_mlp(k):
                g, x, w = gu_t[k], xe_t[k], wd_t[k]
                pk = P[k]
                acts = [apool.tile([P128, pk], BF16, tag="acts", bufs=6,
                                   name=f"a{ii}") for ii in range(NII)]
                for ii in range(NII):
                    for (mo, mw) in _chunks(pk):
                        h1 = ps.tile([P128, mw], F32, tag="acc", name="h1")
                        h2 = ps.tile([P128, mw], F32, tag="acc", name="h2")
                        for h in range(HT):
                            o = h * 2 * I
                            nc.tensor.matmul(
                                h1[:],
                                lhsT=g[:, o + ii * P128:o + (ii + 1) * P128],
                                rhs=x[:, h * pk + mo:h * pk + mo + mw],
                                start=(h == 0), stop=(h == HT - 1))
                        for h in range(HT):
                            o = h * 2 * I
                            nc.tensor.matmul(
                                h2[:],
                                lhsT=g[:, o + I + ii * P128:
                                       o + I + (ii + 1) * P128],
                                rhs=x[:, h * pk + mo:h * pk + mo + mw],
                                start=(h == 0), stop=(h == HT - 1))
                        sl = stpool.tile([P128, mw], F32, tag="sl", name="sl")
                        nc.scalar.activation(sl[:], h1[:], AF.Silu)
                        nc.vector.tensor_mul(acts[ii][:, mo:mo + mw],
                                             sl[:], h2[:])

                # transposed down-proj: out block hb = [128 h, pk tokens]
                ost = stpool.tile([P128, HT * pk], BF16, tag="ost", bufs=2,
                                  name="ost")
                for hb in range(HT):
                    for (mo, mw) in _chunks(pk):
                        dps = ps.tile([P128, mw], F32, tag="acc", name="dps")
                        for ii in range(NII):
                            nc.tensor.matmul(
                                dps[:],
                                lhsT=w[:, ii * H + hb * P128:
                                       ii * H + (hb + 1) * P128],
                                rhs=acts[ii][:, mo:mo + mw],
                                start=(ii == 0), stop=(ii == NII - 1))
                        nc.scalar.activation(
                            ost[:, hb * pk + mo:hb * pk + mo + mw],
                            dps[:], AF.Copy)
                # last slot's output on the scalar HWDGE ring (fast
                # completion right behind its copies); others on gpsimd
                eng = nc.scalar if k == S - 1 else nc.gpsimd
                eng.dma_start(out=ro_d[k][:], in_=ost[:])

            acts_s = [apool.tile([P128, T], BF16, tag="acts_s", bufs=2,
                                 name=f"as{ii}") for ii in range(NIS)]

            def shared_gu():
                # h-major: consume xt h-blocks as they land, 8 PSUM banks
                # hold the (ii, chunk, gate/up) accumulators
                chs = _chunks(T)
                h1 = [[ps.tile([P128, mw], F32, tag="acc", name=f"sg{ii}{ci}")
                       for ci, (mo, mw) in enumerate(chs)]
                      for ii in range(NIS)]
                h2 = [[ps.tile([P128, mw], F32, tag="acc", name=f"su{ii}{ci}")
                       for ci, (mo, mw) in enumerate(chs)]
                      for ii in range(NIS)]
                for h in range(HT):
                    o = h * 2 * ISH
                    for ii in range(NIS):
                        for ci, (mo, mw) in enumerate(chs):
                            nc.tensor.matmul(
                                h1[ii][ci][:],
                                lhsT=wsgu_t[:, o + ii * P128:
                                            o + (ii + 1) * P128],
                                rhs=xt_t[:, h * T + mo:h * T + mo + mw],
                                start=(h == 0), stop=(h == HT - 1))
                            nc.tensor.matmul(
                                h2[ii][ci][:],
                                lhsT=wsgu_t[:, o + ISH + ii * P128:
                                            o + ISH + (ii + 1) * P128],
                                rhs=xt_t[:, h * T + mo:h * T + mo + mw],
                                start=(h == 0), stop=(h == HT - 1))
                for ii in range(NIS):
                    for ci, (mo, mw) in enumerate(chs):
                        sl = stpool.tile([P128, mw], F32, tag="sl", name="sl")
                        nc.scalar.activation(sl[:], h1[ii][ci][:], AF.Silu)
                        nc.vector.tensor_mul(acts_s[ii][:, mo:mo + mw],
                                             sl[:], h2[ii][ci][:])

            def shared_down():
                for half in range(2):
                    ost = stpool.tile([P128, HT * T // 2], BF16, tag="osts",
                                      bufs=2, name="osts")
                    for hb in range(HT // 2):
                        hbb = half * (HT // 2) + hb
                        for (mo, mw) in _chunks(T):
                            dps = ps.tile([P128, mw], F32, tag="acc",
                                          name="dps")
                            for ii in range(NIS):
                                nc.tensor.matmul(
                                    dps[:],
                                    lhsT=wsd_t[:, ii * H + hbb * P128:
                                               ii * H + (hbb + 1) * P128],
                                    rhs=acts_s[ii][:, mo:mo + mw],
                                    start=(ii == 0), stop=(ii == NIS - 1))
                            nc.vector.tensor_copy(
                                ost[:, hb * T + mo:hb * T + mo + mw], dps[:])
                    nc.gpsimd.dma_start(
                        out=so_d[:, half * (HT * T // 2):
                                 (half + 1) * (HT * T // 2)],
                        in_=ost[:])

            shared_gu()
            gated_mlp(0)
            gated_mlp(1)
            gated_mlp(2)
            shared_down()
            gated_mlp(3)
            gated_mlp(4)

    nc.compile()
    return nc


def _prepare(inputs):
    """Host-side dispatch prep: returns (in_maps, P, cells)."""
    x = np.ascontiguousarray(inputs["hidden_states"], dtype=np.float32)
    gate_w = np.asarray(inputs["gate_w"], dtype=np.float32)
    e_bias = np.asarray(inputs["e_bias"], dtype=np.float32)
    w_gate = np.asarray(inputs["w_gate"], dtype=np.float32)
    w_up = np.asarray(inputs["w_up"], dtype=np.float32)
    w_down = np.asarray(inputs["w_down"], dtype=np.float32)
    ws_gate = np.asarray(inputs["ws_gate"], dtype=np.float32)
    ws_up = np.asarray(inputs["ws_up"], dtype=np.float32)
    ws_down = np.asarray(inputs["ws_down"], dtype=np.float32)

    emask, comb = _host_routing(x, gate_w, e_bias)
    counts = emask.sum(0).astype(np.int64)
    tok_lists = [np.nonzero(emask[:, e])[0] for e in range(E)]
    cells = _split_cells(counts, tok_lists)     # len NCORES*S, sorted desc
    grid = [[cells[k * NCORES + c] for c in range(NCORES)] for k in range(S)]
    P = [_pad8(max(len(cell[1]) for cell in tier)) for tier in grid]

    xb = x.astype(BF)
    wgb = w_gate.astype(BF)
    wub = w_up.astype(BF)
    wdb = w_down.astype(BF)

    xt = np.empty((P128, HT * T), dtype=BF)
    for h in range(HT):
        xt[:, h * T:(h + 1) * T] = xb[:, h * P128:(h + 1) * P128].T
    wk = np.zeros((P128, P128), dtype=BF)

    in_maps = []
    for c in range(NCORES):
        wsgu = np.empty((P128, HT * 2 * ISH), dtype=BF)
        for h in range(HT):
            o = h * 2 * ISH
            wsgu[:, o:o + ISH] = \
                ws_gate[h * P128:(h + 1) * P128, c * ISH:(c + 1) * ISH]
            wsgu[:, o + ISH:o + 2 * ISH] = \
                ws_up[h * P128:(h + 1) * P128, c * ISH:(c + 1) * ISH]
        wsd = np.empty((P128, (ISH // P128) * H), dtype=BF)
        for ii in range(ISH // P128):
            wsd[:, ii * H:(ii + 1) * H] = \
                ws_down[c * ISH + ii * P128:c * ISH + (ii + 1) * P128, :]
        m = {"wk": wk, "xt": xt, "wsgu": wsgu, "wsd": wsd}

        for k in range(S):
            e, toks = grid[k][c]
            n = len(toks)
            gu = np.zeros((P128, HT * 2 * I), dtype=BF)
            xp = np.zeros((P128, HT * P[k]), dtype=BF)
            wd = np.zeros((P128, (I // P128) * H), dtype=BF)
            if e is not None:
                xe = xb[toks].T                    # [H, n]
                for h in range(HT):
                    o = h * 2 * I
                    gu[:, o:o + I] = wgb[e, h * P128:(h + 1) * P128, :]
                    gu[:, o + I:o + 2 * I] = wub[e, h * P128:(h + 1) * P128, :]
                    if n:
                        xp[:, h * P[k]:h * P[k] + n] = \
                            xe[h * P128:(h + 1) * P128, :]
                for ii in range(I // P128):
                    wd[:, ii * H:(ii + 1) * H] = \
                        wdb[e, ii * P128:(ii + 1) * P128, :]
            m[f"gu{k}"] = gu
            m[f"xe{k}"] = xp
            m[f"wd{k}"] = wd
        in_maps.append(m)

    return in_maps, P, grid, comb


def _recombine(results, P, grid, comb):
    out = np.zeros((T, H), dtype=np.float32)
    # shared partials: so[p, h*T + t] = partial[t, h*128+p]
    for c in range(NCORES):
        so = np.asarray(results[c]["so"], dtype=np.float32)
        out += so.reshape(P128, HT, T).transpose(2, 1, 0).reshape(T, H)
    # routed: ro[p, hb*P + j] = down_out[token j, hb*128+p]; scale on host
    for c in range(NCORES):
        for k in range(S):
            e, toks = grid[k][c]
            n = len(toks)
            if e is None or n == 0:
                continue
            ro = np.asarray(results[c][f"ro{k}"], dtype=np.float32)
            contrib = ro.reshape(P128, HT, P[k])[:, :, :n]   # [128, HT, n]
            contrib = contrib.transpose(2, 1, 0).reshape(n, H)
            out[toks] += contrib * comb[toks, e][:, None]
    return out


def kernel(**inputs):
    global LAST_RESULTS
    in_maps, P, grid, comb = _prepare(inputs)
    nc = _build_program(P)
    trace = bool(int(os.environ.get("KERNEL_TRACE", "0")))
    if trace:
        trace = _install_ntff_hook()
    LAST_RESULTS = run_bass_kernel_spmd(
        nc, in_maps, list(range(NCORES)), trace=trace)
    results = LAST_RESULTS.results
    return _recombine(results, P, grid, comb)
